# revision 15
# baseline (speedup 1.0000x reference)
"""Disen-GCN (8-channel routing attention GNN) on 8 TRN2 NeuronCores.

Row-parallel sharding: core r owns node rows [r*512, (r+1)*512).
Per routing iteration:
  phase1: L[c][j, i] = z[c,j] . z[c,i]  fp8e4 DoubleRow matmul (K=64
          folded 2-per-partition onto 32 partitions; 256 PE cyc/instr)
  exp:    E8[:, c*512+i] = exp(L)       (ACT, PSUM->SBUF fp16)
  smax:   S = sum_c E8 (v-add on Pool); Q = mask * 1/S (DVE)
  R:      R = E8 * broadcast(Q)  (ch 0-5 DVE, ch 6-7 Pool)
  phase3: agg^T[c][d, i] += znat[c][j,:]^T @ R[c]  (fp16 PE, PSUM acc)
  norm:   z = l2norm(z + agg); re-quantize z to fp8 fold layout
  ship:   one merged AllGather (fp16 znat rows + bitcast fp8 zT fold)
          via internal shared DRAM.
Final: out = concat_c(z) @ W_o + bias.
"""

import numpy as np
from contextlib import ExitStack

from concourse import bacc, bass, tile, mybir
from concourse.bass_utils import run_bass_kernel_spmd
from concourse import dve_ops as _dvo
from concourse.dve_spec import Spec, Src0, Src1, C0, C1, AluOp, Bin
from concourse.dve_spec import lower as _dve_lower
from concourse.dve_ops import DveOp, DveOpSpec


def _ref_qrecip(in0, in1, c0, c1, c2):
    x = np.asarray(in0, dtype=np.float32)
    not_x = (~x.view(np.int32)).view(np.float32)
    y0 = not_x * np.float32(c0)
    y1 = y0 * (np.float32(c1) - x * y0)
    return y1 * np.asarray(in1, dtype=np.float32)


def _make_qrecip():
    # Q = mask * approx(1/S): BITWISE_NOT exponent-flip seed + one
    # Newton-Raphson pass (~0.4% rel err, plenty for fp16 weights),
    # fused with the mask multiply. 6 ALU stages.
    not_x = Bin(AluOp.BITWISE_NOT, Src0, Src0)
    y0 = not_x * C0
    y1 = y0 * (C1 - Src0 * y0)
    spec = Spec(body=y1 * Src1, reference=_ref_qrecip)
    name = "QRECIP_ANT"
    opcode = _dvo._CUSTOM_DVE_ROW_BASE + len(_dvo.OPS)
    assert opcode < 0x20
    shas = {}
    for ver in ("v3", "v4"):
        s = DveOpSpec(name=name, opcode=opcode, uops=_dve_lower(spec, ver=ver),
                      rd1_en=True)
        shas[ver] = s.sha(ver)
    op = DveOp(name, spec, subdim=False, uops_sha=shas,
               perf_en={"v3": True, "v4": True})
    _dvo.OPS.append(op)
    _dvo._SUB_OPCODE_FOR_NAME[name] = opcode
    _dvo.CUSTOM_DVE_SPECS[name] = spec
    return op


QRECIP = _make_qrecip()
QRECIP_C0 = float(_dvo.RECIP_APPROX_FAST_CONSTS["s0"])
QRECIP_C1 = float(_dvo.RECIP_APPROX_FAST_CONSTS["s1"])

F32 = mybir.dt.float32
F16 = mybir.dt.float16
F8 = mybir.dt.float8e4
DR = mybir.MatmulPerfMode.DoubleRow

N = 4096
C = 8
IN_DIM = 256
D = 64
OUT = 128
ITERS = 4
NCORES = 8
NL = N // NCORES          # 512 local rows
CD = C * D                # 512
NJT = N // 128            # 32 j-tiles
NPAIR = C // 2            # 4 channel-pair tiles
AF = mybir.ActivationFunctionType
RG = [list(range(NCORES))]
PIPE_DEPTH = 2            # phase3 lags the softmax by this many j-tiles
E8W = C * NL              # 4096: fused E/R tile width


def _build_nc():
    nc = bacc.Bacc(
        "TRN2", target_bir_lowering=False, debug=False, num_devices=NCORES
    )
    featT = nc.dram_tensor("featT", [IN_DIM, NL], F16, kind="ExternalInput").ap()
    wall = nc.dram_tensor("wall", [IN_DIM, CD], F16, kind="ExternalInput").ap()
    bflat = nc.dram_tensor("bflat", [128, NPAIR], F32, kind="ExternalInput").ap()
    maskT = nc.dram_tensor("maskT", [N, NL], F16, kind="ExternalInput").ap()
    wo = nc.dram_tensor("wo", [CD, OUT], F16, kind="ExternalInput").ap()
    biasd = nc.dram_tensor("biasd", [1, OUT], F16, kind="ExternalInput").ap()
    ident = nc.dram_tensor("ident", [128, 128], F16, kind="ExternalInput").ap()
    blkd = nc.dram_tensor("blkd", [128, NPAIR * 8], F16, kind="ExternalInput").ap()
    seld = nc.dram_tensor("seld", [8, NPAIR * 128], F16, kind="ExternalInput").ap()
    onesd = nc.dram_tensor("onesd", [1, 128], F16, kind="ExternalInput").ap()
    outd = nc.dram_tensor("outd", [NL, OUT], F32, kind="ExternalOutput").ap()

    with tile.TileContext(nc) as tc:
        _body(nc, tc, featT, wall, bflat, maskT, wo, biasd, ident, blkd, seld,
              onesd, outd)
    nc.compile()
    return nc


def _body(nc, tc, featT, wall, bflat, maskT, wo, biasd, ident, blkd, seld,
          onesd, outd):
    ctx = ExitStack()
    const = ctx.enter_context(tc.tile_pool(name="const", bufs=1))
    big = ctx.enter_context(tc.tile_pool(name="big", bufs=1))
    work = ctx.enter_context(tc.tile_pool(name="work", bufs=1))
    psum = ctx.enter_context(tc.tile_pool(name="psum", bufs=1, space="PSUM"))
    dram = ctx.enter_context(tc.tile_pool(name="dram", bufs=1, space="DRAM"))

    def loadc(dr_ap, shape, name):
        dst = const.tile(shape, F16, tag=name, bufs=1, name=name)
        nc.sync.dma_start(out=dst, in_=dr_ap)
        return dst

    # ---- constants / weights (fp16 already on host) ----
    ident16 = loadc(ident, [128, 128], "ident16")
    blkd16 = loadc(blkd, [128, NPAIR * 8], "blkd16")
    sel16 = loadc(seld, [8, NPAIR * 128], "sel16")
    ones16 = loadc(onesd, [1, 128], "ones16")
    bT32 = const.tile([128, NPAIR], F32, tag="bT32", bufs=1, name="bT32")
    nc.sync.dma_start(out=bT32, in_=bflat)
    bias16 = loadc(biasd, [1, OUT], "bias16")
    zeros16 = const.tile([1, NL], F16, tag="zeros16", bufs=1, name="zeros16")
    nc.vector.memset(zeros16, 0.0)

    featT16 = const.tile([128, 2 * NL], F16, tag="featT16", bufs=1, name="featT16")
    nc.sync.dma_start(
        out=featT16.rearrange("p (k i) -> p k i", k=2),
        in_=featT.rearrange("(k p) i -> p k i", p=128))
    w016 = const.tile([128, 2 * CD], F16, tag="w016", bufs=1, name="w016")
    nc.sync.dma_start(
        out=w016.rearrange("p (k i) -> p k i", k=2),
        in_=wall.rearrange("(k p) i -> p k i", p=128))
    wo16 = const.tile([128, 4 * OUT], F16, tag="wo16", bufs=1, name="wo16")
    nc.sync.dma_start(
        out=wo16.rearrange("p (k i) -> p k i", k=4),
        in_=wo.rearrange("(k p) i -> p k i", p=128))

    # ---- resident mask (fp16): mask16[:, jt*512 + i] = adj[i_global, j] ----
    # per-jt DMAs on the gpsimd queue so they don't block the ship DMAs
    mask16 = big.tile([128, NJT * NL], F16, tag="mask16", bufs=1, name="mask16")
    for jt in range(NJT):
        nc.gpsimd.dma_start(
            out=mask16[:, jt * NL:(jt + 1) * NL],
            in_=maskT[jt * 128:(jt + 1) * 128, :])

    # ---- resident full z: fp8 folded zT (phase1) + fp16 natural (phase3) ----
    # zT8f: channel c at partitions [(c%2)*64, +32), cols
    #   (c//2)*8*1024 + r*1024 + s*512 + i  (r=rank, s=fold slot: d=s*32+p)
    zT8f = big.tile([128, 4 * 8 * 1024], F8, tag="zT8f", bufs=1, name="zT8f")
    znat16 = big.tile([128, NJT * CD], F16, tag="znat16", bufs=1, name="znat16")

    def normalize_and_rows(zpre, it, want_nat=True):
        """zpre: 4 SBUF fp16 tiles [128, NL] (z_T rows layout, pre-norm).
        Returns (zrows, natrows): l2-normalized rows in both layouts."""
        nrm = psum.tile([8, NL], F32, tag="L", bufs=2, name=f"nrm_{it}")
        for t in range(NPAIR):
            sq = work.tile([128, NL], F16, tag="sq", bufs=2, name=f"sq_{it}_{t}")
            nc.vector.tensor_mul(out=sq, in0=zpre[t], in1=zpre[t])
            nc.tensor.matmul(out=nrm, lhsT=blkd16[:, t * 8:(t + 1) * 8], rhs=sq,
                             start=(t == 0), stop=(t == NPAIR - 1))
        rsq = work.tile([8, NL], F16, tag="rsq", bufs=2, name=f"rsq_{it}")
        # rsqrt straight from PSUM (sumsq of this data is bounded >> 1e-12,
        # so the reference's clamp is a numeric no-op)
        nc.scalar.activation(out=rsq, in_=nrm, func=AF.Abs_reciprocal_sqrt)
        zrows = []
        for t in range(NPAIR):
            bc = psum.tile([128, NL], F32, tag="L", bufs=2, name=f"bc_{it}_{t}")
            nc.tensor.matmul(out=bc, lhsT=sel16[:, t * 128:(t + 1) * 128],
                             rhs=rsq, start=True, stop=True)
            zr = work.tile([128, NL], F16, tag="zrows", bufs=8,
                           name=f"zrows_{it}_{t}")
            nc.vector.tensor_mul(out=zr, in0=zpre[t], in1=bc)
            zrows.append(zr)
        if not want_nat:
            return zrows, None
        natrows = [work.tile([128, CD], F16, tag="natrows", bufs=4,
                             name=f"natr_{it}_{ib}") for ib in range(4)]
        for t in range(NPAIR):
            for ib in range(4):
                tp = psum.tile([128, 128], F16, tag="L", bufs=2,
                               name=f"tp_{it}_{t}_{ib}")
                nc.tensor.transpose(out=tp,
                                    in_=zrows[t][:, ib * 128:(ib + 1) * 128],
                                    identity=ident16)
                nc.vector.tensor_copy(
                    out=natrows[ib][:, t * 128:(t + 1) * 128], in_=tp)
        return zrows, natrows

    AGR = 768  # rows per rank in the merged AllGather buffer

    def ship_all(zrows, natrows, it):
        """One merged AllGather: rows 0-511 nat fp16, rows 512-767 hold the
        fp8 folded zT (bitcast into the fp16 buffer). Returns the local
        folded rhs tile zfold8: channel c at partitions [(c%2)*64, +32),
        cols (c//2)*1024 + s*512 + i, where z[c, i, d] sits at fold
        partition p=d%32, slot s=d//32."""
        ag_in = dram.tile([AGR, CD], F16, tag="agin", bufs=2,
                          name=f"agin_{it}")
        ag8i = ag_in.bitcast(F8)          # [768, 1024] byte view
        for ib in range(4):
            nc.sync.dma_start(out=ag_in[ib * 128:(ib + 1) * 128, :],
                              in_=natrows[ib])
        for t in range(NPAIR):
            z8 = work.tile([128, NL], F8, tag="z8", bufs=4,
                           name=f"z8_{it}_{t}")
            # fp16 -> fp8 copy on ACT (Copy is in the exp table set)
            nc.scalar.activation(out=z8, in_=zrows[t], func=AF.Copy)
            for h in range(2):
                c = 2 * t + h
                b = c % 2
                cc = c // 2
                # fold row fr=b*32+p, byte j=cc*1024+s*512+i sits at
                # fp16-row 512+fr*4+j//1024, byte-col j%1024
                nc.sync.dma_start(
                    out=ag8i[512 + b * 128:512 + (b + 1) * 128, :]
                        .rearrange("(p four) i -> p four i", four=4)
                        [:, cc, :]
                        .rearrange("p (s i) -> s p i", s=2),
                    in_=z8[h * 64:(h + 1) * 64, :])
        # local folded rhs for phase1 (round-trip through ag_in)
        zfold8 = work.tile([128, 4 * 1024], F8, tag="zfold8", bufs=2,
                           name=f"zfold8_{it}")
        nc.sync.dma_start(
            out=zfold8[0:32, :],
            in_=ag8i[512:640, :].rearrange("(p four) i -> p (four i)", four=4))
        nc.sync.dma_start(
            out=zfold8[64:96, :],
            in_=ag8i[640:768, :].rearrange("(p four) i -> p (four i)", four=4))
        ag_out = dram.tile([NCORES * AGR, CD], F16, tag="agout", bufs=2,
                           addr_space="Shared", name=f"agout_{it}")
        nc.gpsimd.collective_compute(
            "AllGather", mybir.AluOpType.bypass, replica_groups=RG,
            ins=[ag_in.opt()], outs=[ag_out.opt()])
        ag8o = ag_out.bitcast(F8)
        for r in range(NCORES):
            # nat readback: one DMA per rank
            nc.sync.dma_start(
                out=znat16[:, r * 4 * CD:(r + 1) * 4 * CD]
                    .rearrange("p (pb d) -> p pb d", pb=4),
                in_=ag_out[r * AGR:r * AGR + 512, :]
                    .rearrange("(pb p) d -> p pb d", pb=4))
            # fold readback on the gpsimd queue (phase1-critical)
            for b in range(2):
                nc.gpsimd.dma_start(
                    out=zT8f[b * 64:b * 64 + 32, :]
                        .rearrange("p (cc rr f) -> p cc rr f", cc=4, rr=8)
                        [:, :, r, :],
                    in_=ag8o[r * AGR + 512 + b * 128:
                             r * AGR + 512 + (b + 1) * 128, :]
                        .rearrange("(p four) i -> p four i", four=4))
        return zfold8

    # ===== phase 0: z0 = l2norm(features @ W + b), built in z_T layout =====
    zpre0 = []
    for t in range(NPAIR):
        zp = psum.tile([128, NL], F32, tag="L", bufs=2, name=f"zp_{t}")
        for kt in range(2):
            nc.tensor.matmul(
                out=zp,
                lhsT=w016[:, kt * CD + t * 128:kt * CD + (t + 1) * 128],
                rhs=featT16[:, kt * NL:(kt + 1) * NL],
                start=(kt == 0), stop=(kt == 1))
        zt = work.tile([128, NL], F16, tag="zpre0", bufs=5, name=f"zpre0_{t}")
        nc.scalar.activation(out=zt, in_=zp, func=AF.Identity,
                             bias=bT32[:, t:t + 1])
        zpre0.append(zt)
    zrows, natrows = normalize_and_rows(zpre0, it=-1)
    zfold8 = ship_all(zrows, natrows, it=-1)

    # ================= routing iterations =================
    for it in range(ITERS):
        agg = [psum.tile([128, NL], F32, tag="agg", bufs=4, name=f"agg_{it}_{t}")
               for t in range(NPAIR)]
        for t in range(NPAIR):
            # zero-fill the whole bank once so both col-tiled halves can
            # accumulate with start=False (start clears the full bank)
            nc.tensor.matmul(out=agg[t], lhsT=ones16, rhs=zeros16,
                             start=True, stop=False)
        pending = []
        for jt in range(NJT):
            E8 = work.tile([128, E8W], F16, tag="E", bufs=3,
                           name=f"E8_{it}_{jt}")
            for t in range(NPAIR):
                L2 = psum.tile([128, 2 * NL], F32, tag="L", bufs=2,
                               name=f"L2_{it}_{jt}_{t}")
                for h in range(2):
                    c = 2 * t + h
                    cb = (c % 2) * 64       # partition base
                    cc = c // 2             # column block
                    nc.tensor.matmul(
                        out=L2[:, h * NL:(h + 1) * NL],
                        lhsT=zT8f[cb:cb + 32,
                                  cc * 8192 + 0:cc * 8192 + 8192]
                            .rearrange("p (rr s i) -> p rr s i", rr=8, s=2)
                            [:, jt // 4, :, (jt % 4) * 128:(jt % 4 + 1) * 128],
                        rhs=zfold8[cb:cb + 32, cc * 1024:(cc + 1) * 1024]
                            .rearrange("p (s i) -> p s i", s=2),
                        start=True, stop=True, perf_mode=DR,
                        tile_position=(cb, 0))
                nc.scalar.activation(
                    out=E8[:, t * 2 * NL:(t + 1) * 2 * NL], in_=L2,
                    func=AF.Exp)
            # channel-softmax denominator: tree sum split across DVE/Pool
            u = work.tile([128, 2 * NL], F16, tag="s2", bufs=4,
                          name=f"u_{it}_{jt}")
            nc.vector.tensor_add(out=u, in0=E8[:, 0:1024], in1=E8[:, 1024:2048])
            v = work.tile([128, 2 * NL], F16, tag="s2p", bufs=2,
                          name=f"v_{it}_{jt}")
            nc.gpsimd.tensor_add(out=v, in0=E8[:, 2048:3072],
                                 in1=E8[:, 3072:4096])
            w = work.tile([128, 2 * NL], F16, tag="s2", bufs=4,
                          name=f"w_{it}_{jt}")
            nc.vector.tensor_add(out=w, in0=u, in1=v)
            S16 = work.tile([128, NL], F16, tag="S16", bufs=4,
                            name=f"S16_{it}_{jt}")
            nc.vector.tensor_add(out=S16, in0=w[:, 0:NL], in1=w[:, NL:])
            # Q = mask * 1/S in one fused custom-DVE op
            Q = work.tile([128, NL], F16, tag="Q", bufs=4, name=f"Q_{it}_{jt}")
            nc.vector._custom_dve(
                QRECIP, out=Q, in0=S16,
                in1=mask16[:, jt * NL:(jt + 1) * NL],
                s0=QRECIP_C0, s1=QRECIP_C1)
            # R = E * broadcast(Q): channels 0-5 on DVE, 6-7 on GpSimd
            R6 = work.tile([128, 6 * NL], F16, tag="R", bufs=PIPE_DEPTH + 1,
                           name=f"R6_{it}_{jt}")
            nc.vector.tensor_mul(
                out=R6.rearrange("p (c i) -> p c i", c=6),
                in0=E8[:, 0:6 * NL].rearrange("p (c i) -> p c i", c=6),
                in1=Q.unsqueeze(1).broadcast_to([128, 6, NL]))
            R2 = work.tile([128, 2 * NL], F16, tag="Rb", bufs=PIPE_DEPTH + 1,
                           name=f"R2_{it}_{jt}")
            nc.gpsimd.tensor_mul(
                out=R2.rearrange("p (c i) -> p c i", c=2),
                in0=E8[:, 6 * NL:].rearrange("p (c i) -> p c i", c=2),
                in1=Q.unsqueeze(1).broadcast_to([128, 2, NL]))
            pending.append((jt, R6, R2))
            if len(pending) > PIPE_DEPTH:
                pjt, pR6, pR2 = pending.pop(0)
                for c in range(C):
                    t, h = c // 2, c % 2
                    rhs = (pR6[:, c * NL:(c + 1) * NL] if c < 6
                           else pR2[:, (c - 6) * NL:(c - 5) * NL])
                    nc.tensor.matmul(
                        out=agg[t][h * 64:(h + 1) * 64, :],
                        lhsT=znat16[:, pjt * CD + c * 64:pjt * CD + (c + 1) * 64],
                        rhs=rhs,
                        start=False, stop=False,
                        tile_position=(0, h * 64))
        for pjt, pR6, pR2 in pending:
            for c in range(C):
                t, h = c // 2, c % 2
                rhs = (pR6[:, c * NL:(c + 1) * NL] if c < 6
                       else pR2[:, (c - 6) * NL:(c - 5) * NL])
                nc.tensor.matmul(
                    out=agg[t][h * 64:(h + 1) * 64, :],
                    lhsT=znat16[:, pjt * CD + c * 64:pjt * CD + (c + 1) * 64],
                    rhs=rhs,
                    start=False, stop=False,
                    tile_position=(0, h * 64))
        for t in range(NPAIR):
            # N=1 dummy stop: closes the sim accumulation group, no-op on HW
            nc.tensor.matmul(out=agg[t][:, 0:1], lhsT=ones16,
                             rhs=zeros16[:, 0:1], start=False, stop=True)
        # residual + renorm
        zpre = []
        for t in range(NPAIR):
            zq = work.tile([128, NL], F16, tag="zpre0", bufs=5,
                           name=f"zpre_{it}_{t}")
            nc.vector.tensor_add(out=zq, in0=zrows[t], in1=agg[t])
            zpre.append(zq)
        zrows, natrows = normalize_and_rows(zpre, it=it,
                                            want_nat=(it < ITERS - 1))
        if it < ITERS - 1:
            zfold8 = ship_all(zrows, natrows, it=it)

    # ================= output: h @ W_o + bias =================
    for ib in range(4):
        op = psum.tile([128, OUT], F32, tag="L", bufs=2, name=f"op_{ib}")
        for kt in range(4):
            nc.tensor.matmul(out=op,
                             lhsT=zrows[kt][:, ib * 128:(ib + 1) * 128],
                             rhs=wo16[:, kt * OUT:(kt + 1) * OUT],
                             start=(kt == 0), stop=False)
        nc.tensor.matmul(out=op, lhsT=ones16, rhs=bias16, start=False, stop=True)
        ot = work.tile([128, OUT], F32, tag="ot", bufs=2, name=f"ot_{ib}")
        nc.vector.tensor_copy(out=ot, in_=op)
        nc.sync.dma_start(out=outd[ib * 128:(ib + 1) * 128, :], in_=ot)

    ctx.close()


def _make_in_maps(features, adj, W, b, W_o, bias):
    import ml_dtypes
    features = np.asarray(features, dtype=np.float32)
    adj = np.asarray(adj, dtype=np.float32)
    W = np.asarray(W, dtype=np.float32)
    b = np.asarray(b, dtype=np.float32)
    W_o = np.asarray(W_o, dtype=np.float32)
    bias = np.asarray(bias, dtype=np.float32)

    f16 = np.float16
    f8 = ml_dtypes.float8_e4m3
    wall = np.ascontiguousarray(
        W.transpose(1, 0, 2).reshape(IN_DIM, CD)).astype(f16)
    bflat = np.ascontiguousarray(b.reshape(1, CD).reshape(NPAIR, 128).T).astype(np.float32)
    ident = np.eye(128, dtype=f16)
    blkd = np.zeros((128, NPAIR * 8), dtype=f16)
    seld = np.zeros((8, NPAIR * 128), dtype=f16)
    for t in range(NPAIR):
        for h in range(2):
            c = 2 * t + h
            blkd[h * 64:(h + 1) * 64, t * 8 + c] = 1.0
            seld[c, t * 128 + h * 64:t * 128 + (h + 1) * 64] = 1.0
    onesd = np.ones((1, 128), dtype=f16)
    wo16 = W_o.astype(f16)
    bias16 = bias.reshape(1, OUT).astype(f16)

    in_maps = []
    for r in range(NCORES):
        rows = slice(r * NL, (r + 1) * NL)
        in_maps.append({
            "featT": np.ascontiguousarray(features[rows].T).astype(f16),
            "wall": wall,
            "bflat": bflat,
            "maskT": np.ascontiguousarray(adj[rows].T).astype(f16),
            "wo": wo16,
            "biasd": bias16,
            "ident": ident,
            "blkd": blkd,
            "seld": seld,
            "onesd": onesd,
        })
    return in_maps


_NC_CACHE = []


def _get_nc():
    if not _NC_CACHE:
        _NC_CACHE.append(_build_nc())
    return _NC_CACHE[0]


def run(inputs, trace=False, **kwargs):
    nc = _get_nc()
    in_maps = _make_in_maps(**inputs)
    res = run_bass_kernel_spmd(nc, in_maps, core_ids=list(range(NCORES)),
                               trace=trace, **kwargs)
    out = np.concatenate([res.results[r]["outd"] for r in range(NCORES)],
                         axis=0).astype(np.float32)
    return out, res


def kernel(features, adj, W, b, W_o, bias):
    out, _ = run(dict(features=features, adj=adj, W=W, b=b, W_o=W_o, bias=bias))
    return out


# revision 16
# speedup vs baseline: 1.0164x; 1.0164x over previous
"""Disen-GCN (8-channel routing attention GNN) on 8 TRN2 NeuronCores.

Row-parallel sharding: core r owns node rows [r*512, (r+1)*512).
Per routing iteration:
  phase1: L[c][j, i] = z[c,j] . z[c,i]  fp8e4 DoubleRow matmul (K=64
          folded 2-per-partition onto 32 partitions; 256 PE cyc/instr)
  exp:    E8[:, c*512+i] = exp(L)       (ACT, PSUM->SBUF fp16)
  smax:   S = sum_c E8 (v-add on Pool); Q = mask * 1/S (DVE)
  R:      R = E8 * broadcast(Q)  (ch 0-5 DVE, ch 6-7 Pool)
  phase3: agg^T[c][d, i] += znat[c][j,:]^T @ R[c]  (fp16 PE, PSUM acc)
  norm:   z = l2norm(z + agg); re-quantize z to fp8 fold layout
  ship:   one merged AllGather (fp16 znat rows + bitcast fp8 zT fold)
          via internal shared DRAM.
Final: out = concat_c(z) @ W_o + bias.
"""

import numpy as np
from contextlib import ExitStack

from concourse import bacc, bass, tile, mybir
from concourse.bass_utils import run_bass_kernel_spmd
from concourse import dve_ops as _dvo
from concourse.dve_spec import Spec, Src0, Src1, C0, C1, AluOp, Bin
from concourse.dve_spec import lower as _dve_lower
from concourse.dve_ops import DveOp, DveOpSpec


def _ref_qrecip(in0, in1, c0, c1, c2):
    x = np.asarray(in0, dtype=np.float32)
    not_x = (~x.view(np.int32)).view(np.float32)
    y0 = not_x * np.float32(c0)
    y1 = y0 * (np.float32(c1) - x * y0)
    return y1 * np.asarray(in1, dtype=np.float32)


def _make_qrecip():
    # Q = mask * approx(1/S): BITWISE_NOT exponent-flip seed + one
    # Newton-Raphson pass (~0.4% rel err, plenty for fp16 weights),
    # fused with the mask multiply. 6 ALU stages.
    not_x = Bin(AluOp.BITWISE_NOT, Src0, Src0)
    y0 = not_x * C0
    y1 = y0 * (C1 - Src0 * y0)
    spec = Spec(body=y1 * Src1, reference=_ref_qrecip)
    name = "QRECIP_ANT"
    opcode = _dvo._CUSTOM_DVE_ROW_BASE + len(_dvo.OPS)
    assert opcode < 0x20
    shas = {}
    for ver in ("v3", "v4"):
        s = DveOpSpec(name=name, opcode=opcode, uops=_dve_lower(spec, ver=ver),
                      rd1_en=True)
        shas[ver] = s.sha(ver)
    op = DveOp(name, spec, subdim=False, uops_sha=shas,
               perf_en={"v3": True, "v4": True})
    _dvo.OPS.append(op)
    _dvo._SUB_OPCODE_FOR_NAME[name] = opcode
    _dvo.CUSTOM_DVE_SPECS[name] = spec
    return op


QRECIP = _make_qrecip()
QRECIP_C0 = float(_dvo.RECIP_APPROX_FAST_CONSTS["s0"])
QRECIP_C1 = float(_dvo.RECIP_APPROX_FAST_CONSTS["s1"])

F32 = mybir.dt.float32
F16 = mybir.dt.float16
F8 = mybir.dt.float8e4
DR = mybir.MatmulPerfMode.DoubleRow

N = 4096
C = 8
IN_DIM = 256
D = 64
OUT = 128
ITERS = 4
NCORES = 8
NL = N // NCORES          # 512 local rows
CD = C * D                # 512
NJT = N // 128            # 32 j-tiles
NPAIR = C // 2            # 4 channel-pair tiles
AF = mybir.ActivationFunctionType
RG = [list(range(NCORES))]
PIPE_DEPTH = 2            # phase3 lags the softmax by this many j-tiles
E8W = C * NL              # 4096: fused E/R tile width


def _build_nc():
    nc = bacc.Bacc(
        "TRN2", target_bir_lowering=False, debug=False, num_devices=NCORES
    )
    featT = nc.dram_tensor("featT", [IN_DIM, NL], F16, kind="ExternalInput").ap()
    wall = nc.dram_tensor("wall", [IN_DIM, CD], F16, kind="ExternalInput").ap()
    bflat = nc.dram_tensor("bflat", [128, NPAIR], F32, kind="ExternalInput").ap()
    maskT = nc.dram_tensor("maskT", [N, NL], F16, kind="ExternalInput").ap()
    wo = nc.dram_tensor("wo", [CD, OUT], F16, kind="ExternalInput").ap()
    biasd = nc.dram_tensor("biasd", [1, OUT], F16, kind="ExternalInput").ap()
    ident = nc.dram_tensor("ident", [128, 128], F16, kind="ExternalInput").ap()
    blkd = nc.dram_tensor("blkd", [128, NPAIR * 8], F16, kind="ExternalInput").ap()
    seld = nc.dram_tensor("seld", [8, NPAIR * 128], F16, kind="ExternalInput").ap()
    onesd = nc.dram_tensor("onesd", [1, 128], F16, kind="ExternalInput").ap()
    outd = nc.dram_tensor("outd", [NL, OUT], F32, kind="ExternalOutput").ap()

    with tile.TileContext(nc) as tc:
        _body(nc, tc, featT, wall, bflat, maskT, wo, biasd, ident, blkd, seld,
              onesd, outd)
    nc.compile()
    return nc


def _body(nc, tc, featT, wall, bflat, maskT, wo, biasd, ident, blkd, seld,
          onesd, outd):
    ctx = ExitStack()
    const = ctx.enter_context(tc.tile_pool(name="const", bufs=1))
    big = ctx.enter_context(tc.tile_pool(name="big", bufs=1))
    work = ctx.enter_context(tc.tile_pool(name="work", bufs=1))
    psum = ctx.enter_context(tc.tile_pool(name="psum", bufs=1, space="PSUM"))
    dram = ctx.enter_context(tc.tile_pool(name="dram", bufs=1, space="DRAM"))

    def loadc(dr_ap, shape, name):
        dst = const.tile(shape, F16, tag=name, bufs=1, name=name)
        nc.sync.dma_start(out=dst, in_=dr_ap)
        return dst

    # ---- constants / weights (fp16 already on host) ----
    ident16 = loadc(ident, [128, 128], "ident16")
    blkd16 = loadc(blkd, [128, NPAIR * 8], "blkd16")
    sel16 = loadc(seld, [8, NPAIR * 128], "sel16")
    ones16 = loadc(onesd, [1, 128], "ones16")
    bT32 = const.tile([128, NPAIR], F32, tag="bT32", bufs=1, name="bT32")
    nc.sync.dma_start(out=bT32, in_=bflat)
    bias16 = loadc(biasd, [1, OUT], "bias16")
    zeros16 = const.tile([1, NL], F16, tag="zeros16", bufs=1, name="zeros16")
    nc.vector.memset(zeros16, 0.0)

    featT16 = const.tile([128, 2 * NL], F16, tag="featT16", bufs=1, name="featT16")
    nc.sync.dma_start(
        out=featT16.rearrange("p (k i) -> p k i", k=2),
        in_=featT.rearrange("(k p) i -> p k i", p=128))
    w016 = const.tile([128, 2 * CD], F16, tag="w016", bufs=1, name="w016")
    nc.sync.dma_start(
        out=w016.rearrange("p (k i) -> p k i", k=2),
        in_=wall.rearrange("(k p) i -> p k i", p=128))
    wo16 = const.tile([128, 4 * OUT], F16, tag="wo16", bufs=1, name="wo16")
    nc.sync.dma_start(
        out=wo16.rearrange("p (k i) -> p k i", k=4),
        in_=wo.rearrange("(k p) i -> p k i", p=128))

    # ---- resident mask (fp16): mask16[:, jt*512 + i] = adj[i_global, j] ----
    # per-jt DMAs on the gpsimd queue so they don't block the ship DMAs
    mask16 = big.tile([128, NJT * NL], F16, tag="mask16", bufs=1, name="mask16")
    for jt in range(NJT):
        nc.gpsimd.dma_start(
            out=mask16[:, jt * NL:(jt + 1) * NL],
            in_=maskT[jt * 128:(jt + 1) * 128, :])

    # ---- skew-absorbing barrier: a tiny AllGather so the first real
    # collective doesn't pay the cross-core launch skew ----
    sk_in = dram.tile([1, 64], F16, tag="skin", bufs=1, name="sk_in")
    sk_src = const.tile([1, 64], F16, tag="sksrc", bufs=1, name="sk_src")
    nc.vector.memset(sk_src, 0.0)
    nc.sync.dma_start(out=sk_in, in_=sk_src)
    sk_out = dram.tile([NCORES, 64], F16, tag="skout", bufs=1,
                       addr_space="Shared", name="sk_out")
    nc.gpsimd.collective_compute(
        "AllGather", mybir.AluOpType.bypass, replica_groups=RG,
        ins=[sk_in.opt()], outs=[sk_out.opt()])

    # ---- resident full z: fp8 folded zT (phase1) + fp16 natural (phase3) ----
    # zT8f: channel c at partitions [(c%2)*64, +32), cols
    #   (c//2)*8*1024 + r*1024 + s*512 + i  (r=rank, s=fold slot: d=s*32+p)
    zT8f = big.tile([128, 4 * 8 * 1024], F8, tag="zT8f", bufs=1, name="zT8f")
    znat16 = big.tile([128, NJT * CD], F16, tag="znat16", bufs=1, name="znat16")

    def normalize_and_rows(zpre, it, want_nat=True):
        """zpre: 4 SBUF fp16 tiles [128, NL] (z_T rows layout, pre-norm).
        Returns (zrows, natrows): l2-normalized rows in both layouts."""
        nrm = psum.tile([8, NL], F32, tag="L", bufs=2, name=f"nrm_{it}")
        for t in range(NPAIR):
            sq = work.tile([128, NL], F16, tag="sq", bufs=2, name=f"sq_{it}_{t}")
            nc.vector.tensor_mul(out=sq, in0=zpre[t], in1=zpre[t])
            nc.tensor.matmul(out=nrm, lhsT=blkd16[:, t * 8:(t + 1) * 8], rhs=sq,
                             start=(t == 0), stop=(t == NPAIR - 1))
        rsq = work.tile([8, NL], F16, tag="rsq", bufs=2, name=f"rsq_{it}")
        # rsqrt straight from PSUM (sumsq of this data is bounded >> 1e-12,
        # so the reference's clamp is a numeric no-op)
        nc.scalar.activation(out=rsq, in_=nrm, func=AF.Abs_reciprocal_sqrt)
        zrows = []
        for t in range(NPAIR):
            bc = psum.tile([128, NL], F32, tag="L", bufs=2, name=f"bc_{it}_{t}")
            nc.tensor.matmul(out=bc, lhsT=sel16[:, t * 128:(t + 1) * 128],
                             rhs=rsq, start=True, stop=True)
            zr = work.tile([128, NL], F16, tag="zrows", bufs=8,
                           name=f"zrows_{it}_{t}")
            nc.vector.tensor_mul(out=zr, in0=zpre[t], in1=bc)
            zrows.append(zr)
        if not want_nat:
            return zrows, None
        natrows = [work.tile([128, CD], F16, tag="natrows", bufs=4,
                             name=f"natr_{it}_{ib}") for ib in range(4)]
        for t in range(NPAIR):
            for ib in range(4):
                tp = psum.tile([128, 128], F16, tag="L", bufs=2,
                               name=f"tp_{it}_{t}_{ib}")
                nc.tensor.transpose(out=tp,
                                    in_=zrows[t][:, ib * 128:(ib + 1) * 128],
                                    identity=ident16)
                nc.vector.tensor_copy(
                    out=natrows[ib][:, t * 128:(t + 1) * 128], in_=tp)
        return zrows, natrows

    AGR = 768  # rows per rank in the merged AllGather buffer

    def ship_all(zrows, natrows, it):
        """One merged AllGather: rows 0-511 nat fp16, rows 512-767 hold the
        fp8 folded zT (bitcast into the fp16 buffer). Returns the local
        folded rhs tile zfold8: channel c at partitions [(c%2)*64, +32),
        cols (c//2)*1024 + s*512 + i, where z[c, i, d] sits at fold
        partition p=d%32, slot s=d//32."""
        ag_in = dram.tile([AGR, CD], F16, tag="agin", bufs=2,
                          name=f"agin_{it}")
        ag8i = ag_in.bitcast(F8)          # [768, 1024] byte view
        for ib in range(4):
            nc.sync.dma_start(out=ag_in[ib * 128:(ib + 1) * 128, :],
                              in_=natrows[ib])
        for t in range(NPAIR):
            z8 = work.tile([128, NL], F8, tag="z8", bufs=4,
                           name=f"z8_{it}_{t}")
            # fp16 -> fp8 copy on ACT (Copy is in the exp table set)
            nc.scalar.activation(out=z8, in_=zrows[t], func=AF.Copy)
            for h in range(2):
                c = 2 * t + h
                b = c % 2
                cc = c // 2
                # fold row fr=b*32+p, byte j=cc*1024+s*512+i sits at
                # fp16-row 512+fr*4+j//1024, byte-col j%1024
                nc.sync.dma_start(
                    out=ag8i[512 + b * 128:512 + (b + 1) * 128, :]
                        .rearrange("(p four) i -> p four i", four=4)
                        [:, cc, :]
                        .rearrange("p (s i) -> s p i", s=2),
                    in_=z8[h * 64:(h + 1) * 64, :])
        # local folded rhs for phase1 (round-trip through ag_in)
        zfold8 = work.tile([128, 4 * 1024], F8, tag="zfold8", bufs=2,
                           name=f"zfold8_{it}")
        nc.sync.dma_start(
            out=zfold8[0:32, :],
            in_=ag8i[512:640, :].rearrange("(p four) i -> p (four i)", four=4))
        nc.sync.dma_start(
            out=zfold8[64:96, :],
            in_=ag8i[640:768, :].rearrange("(p four) i -> p (four i)", four=4))
        ag_out = dram.tile([NCORES * AGR, CD], F16, tag="agout", bufs=2,
                           addr_space="Shared", name=f"agout_{it}")
        nc.gpsimd.collective_compute(
            "AllGather", mybir.AluOpType.bypass, replica_groups=RG,
            ins=[ag_in.opt()], outs=[ag_out.opt()])
        ag8o = ag_out.bitcast(F8)
        for r in range(NCORES):
            # nat readback: one DMA per rank
            nc.sync.dma_start(
                out=znat16[:, r * 4 * CD:(r + 1) * 4 * CD]
                    .rearrange("p (pb d) -> p pb d", pb=4),
                in_=ag_out[r * AGR:r * AGR + 512, :]
                    .rearrange("(pb p) d -> p pb d", pb=4))
            # fold readback on the gpsimd queue (phase1-critical)
            for b in range(2):
                nc.gpsimd.dma_start(
                    out=zT8f[b * 64:b * 64 + 32, :]
                        .rearrange("p (cc rr f) -> p cc rr f", cc=4, rr=8)
                        [:, :, r, :],
                    in_=ag8o[r * AGR + 512 + b * 128:
                             r * AGR + 512 + (b + 1) * 128, :]
                        .rearrange("(p four) i -> p four i", four=4))
        return zfold8

    # ===== phase 0: z0 = l2norm(features @ W + b), built in z_T layout =====
    zpre0 = []
    for t in range(NPAIR):
        zp = psum.tile([128, NL], F32, tag="L", bufs=2, name=f"zp_{t}")
        for kt in range(2):
            nc.tensor.matmul(
                out=zp,
                lhsT=w016[:, kt * CD + t * 128:kt * CD + (t + 1) * 128],
                rhs=featT16[:, kt * NL:(kt + 1) * NL],
                start=(kt == 0), stop=(kt == 1))
        zt = work.tile([128, NL], F16, tag="zpre0", bufs=5, name=f"zpre0_{t}")
        nc.scalar.activation(out=zt, in_=zp, func=AF.Identity,
                             bias=bT32[:, t:t + 1])
        zpre0.append(zt)
    zrows, natrows = normalize_and_rows(zpre0, it=-1)
    zfold8 = ship_all(zrows, natrows, it=-1)

    # ================= routing iterations =================
    for it in range(ITERS):
        agg = [psum.tile([128, NL], F32, tag="agg", bufs=4, name=f"agg_{it}_{t}")
               for t in range(NPAIR)]
        for t in range(NPAIR):
            # zero-fill the whole bank once so both col-tiled halves can
            # accumulate with start=False (start clears the full bank)
            nc.tensor.matmul(out=agg[t], lhsT=ones16, rhs=zeros16,
                             start=True, stop=False)
        pending = []
        for jt in range(NJT):
            E8 = work.tile([128, E8W], F16, tag="E", bufs=3,
                           name=f"E8_{it}_{jt}")
            for t in range(NPAIR):
                L2 = psum.tile([128, 2 * NL], F32, tag="L", bufs=2,
                               name=f"L2_{it}_{jt}_{t}")
                for h in range(2):
                    c = 2 * t + h
                    cb = (c % 2) * 64       # partition base
                    cc = c // 2             # column block
                    nc.tensor.matmul(
                        out=L2[:, h * NL:(h + 1) * NL],
                        lhsT=zT8f[cb:cb + 32,
                                  cc * 8192 + 0:cc * 8192 + 8192]
                            .rearrange("p (rr s i) -> p rr s i", rr=8, s=2)
                            [:, jt // 4, :, (jt % 4) * 128:(jt % 4 + 1) * 128],
                        rhs=zfold8[cb:cb + 32, cc * 1024:(cc + 1) * 1024]
                            .rearrange("p (s i) -> p s i", s=2),
                        start=True, stop=True, perf_mode=DR,
                        tile_position=(cb, 0))
                nc.scalar.activation(
                    out=E8[:, t * 2 * NL:(t + 1) * 2 * NL], in_=L2,
                    func=AF.Exp)
            # channel-softmax denominator: first-level adds on Pool
            u = work.tile([128, 2 * NL], F16, tag="s2p", bufs=3,
                          name=f"u_{it}_{jt}")
            nc.gpsimd.tensor_add(out=u, in0=E8[:, 0:1024], in1=E8[:, 1024:2048])
            v = work.tile([128, 2 * NL], F16, tag="s2p", bufs=3,
                          name=f"v_{it}_{jt}")
            nc.gpsimd.tensor_add(out=v, in0=E8[:, 2048:3072],
                                 in1=E8[:, 3072:4096])
            w = work.tile([128, 2 * NL], F16, tag="s2", bufs=3,
                          name=f"w_{it}_{jt}")
            nc.vector.tensor_add(out=w, in0=u, in1=v)
            S16 = work.tile([128, NL], F16, tag="S16", bufs=4,
                            name=f"S16_{it}_{jt}")
            nc.vector.tensor_add(out=S16, in0=w[:, 0:NL], in1=w[:, NL:])
            # Q = mask * 1/S in one fused custom-DVE op
            Q = work.tile([128, NL], F16, tag="Q", bufs=4, name=f"Q_{it}_{jt}")
            nc.vector._custom_dve(
                QRECIP, out=Q, in0=S16,
                in1=mask16[:, jt * NL:(jt + 1) * NL],
                s0=QRECIP_C0, s1=QRECIP_C1)
            # R8 = E8 * broadcast(Q) over the 8 channel blocks, one DVE op
            R8 = work.tile([128, E8W], F16, tag="R", bufs=PIPE_DEPTH + 1,
                           name=f"R8_{it}_{jt}")
            nc.vector.tensor_mul(
                out=R8.rearrange("p (c i) -> p c i", c=C),
                in0=E8.rearrange("p (c i) -> p c i", c=C),
                in1=Q.unsqueeze(1).broadcast_to([128, C, NL]))
            pending.append((jt, R8))
            if len(pending) > PIPE_DEPTH:
                pjt, pR8 = pending.pop(0)
                for c in range(C):
                    t, h = c // 2, c % 2
                    nc.tensor.matmul(
                        out=agg[t][h * 64:(h + 1) * 64, :],
                        lhsT=znat16[:, pjt * CD + c * 64:pjt * CD + (c + 1) * 64],
                        rhs=pR8[:, c * NL:(c + 1) * NL],
                        start=False, stop=False,
                        tile_position=(0, h * 64))
        for pjt, pR8 in pending:
            for c in range(C):
                t, h = c // 2, c % 2
                nc.tensor.matmul(
                    out=agg[t][h * 64:(h + 1) * 64, :],
                    lhsT=znat16[:, pjt * CD + c * 64:pjt * CD + (c + 1) * 64],
                    rhs=pR8[:, c * NL:(c + 1) * NL],
                    start=False, stop=False,
                    tile_position=(0, h * 64))
        for t in range(NPAIR):
            # N=1 dummy stop: closes the sim accumulation group, no-op on HW
            nc.tensor.matmul(out=agg[t][:, 0:1], lhsT=ones16,
                             rhs=zeros16[:, 0:1], start=False, stop=True)
        # residual + renorm
        zpre = []
        for t in range(NPAIR):
            zq = work.tile([128, NL], F16, tag="zpre0", bufs=5,
                           name=f"zpre_{it}_{t}")
            nc.vector.tensor_add(out=zq, in0=zrows[t], in1=agg[t])
            zpre.append(zq)
        zrows, natrows = normalize_and_rows(zpre, it=it,
                                            want_nat=(it < ITERS - 1))
        if it < ITERS - 1:
            zfold8 = ship_all(zrows, natrows, it=it)

    # ================= output: h @ W_o + bias =================
    for ib in range(4):
        op = psum.tile([128, OUT], F32, tag="L", bufs=2, name=f"op_{ib}")
        for kt in range(4):
            nc.tensor.matmul(out=op,
                             lhsT=zrows[kt][:, ib * 128:(ib + 1) * 128],
                             rhs=wo16[:, kt * OUT:(kt + 1) * OUT],
                             start=(kt == 0), stop=False)
        nc.tensor.matmul(out=op, lhsT=ones16, rhs=bias16, start=False, stop=True)
        ot = work.tile([128, OUT], F32, tag="ot", bufs=2, name=f"ot_{ib}")
        nc.vector.tensor_copy(out=ot, in_=op)
        nc.sync.dma_start(out=outd[ib * 128:(ib + 1) * 128, :], in_=ot)

    ctx.close()


def _make_in_maps(features, adj, W, b, W_o, bias):
    import ml_dtypes
    features = np.asarray(features, dtype=np.float32)
    adj = np.asarray(adj, dtype=np.float32)
    W = np.asarray(W, dtype=np.float32)
    b = np.asarray(b, dtype=np.float32)
    W_o = np.asarray(W_o, dtype=np.float32)
    bias = np.asarray(bias, dtype=np.float32)

    f16 = np.float16
    f8 = ml_dtypes.float8_e4m3
    wall = np.ascontiguousarray(
        W.transpose(1, 0, 2).reshape(IN_DIM, CD)).astype(f16)
    bflat = np.ascontiguousarray(b.reshape(1, CD).reshape(NPAIR, 128).T).astype(np.float32)
    ident = np.eye(128, dtype=f16)
    blkd = np.zeros((128, NPAIR * 8), dtype=f16)
    seld = np.zeros((8, NPAIR * 128), dtype=f16)
    for t in range(NPAIR):
        for h in range(2):
            c = 2 * t + h
            blkd[h * 64:(h + 1) * 64, t * 8 + c] = 1.0
            seld[c, t * 128 + h * 64:t * 128 + (h + 1) * 64] = 1.0
    onesd = np.ones((1, 128), dtype=f16)
    wo16 = W_o.astype(f16)
    bias16 = bias.reshape(1, OUT).astype(f16)

    in_maps = []
    for r in range(NCORES):
        rows = slice(r * NL, (r + 1) * NL)
        in_maps.append({
            "featT": np.ascontiguousarray(features[rows].T).astype(f16),
            "wall": wall,
            "bflat": bflat,
            "maskT": np.ascontiguousarray(adj[rows].T).astype(f16),
            "wo": wo16,
            "biasd": bias16,
            "ident": ident,
            "blkd": blkd,
            "seld": seld,
            "onesd": onesd,
        })
    return in_maps


_NC_CACHE = []


def _get_nc():
    if not _NC_CACHE:
        _NC_CACHE.append(_build_nc())
    return _NC_CACHE[0]


def run(inputs, trace=False, **kwargs):
    nc = _get_nc()
    in_maps = _make_in_maps(**inputs)
    res = run_bass_kernel_spmd(nc, in_maps, core_ids=list(range(NCORES)),
                               trace=trace, **kwargs)
    out = np.concatenate([res.results[r]["outd"] for r in range(NCORES)],
                         axis=0).astype(np.float32)
    return out, res


def kernel(features, adj, W, b, W_o, bias):
    out, _ = run(dict(features=features, adj=adj, W=W, b=b, W_o=W_o, bias=bias))
    return out


# revision 17
# speedup vs baseline: 1.2948x; 1.2739x over previous
"""Disen-GCN (8-channel routing attention GNN) on 8 TRN2 NeuronCores.

Row-parallel sharding: core r owns node rows [r*512, (r+1)*512).
Per routing iteration:
  phase1: L[c][j, i] = z[c,j] . z[c,i]  fp8e4 DoubleRow matmul (K=64
          folded 2-per-partition onto 32 partitions; 256 PE cyc/instr)
  exp:    E8[:, c*512+i] = exp(L)       (ACT, PSUM->SBUF fp16)
  smax:   S = sum_c E8 (v-add on Pool); Q = mask * 1/S (DVE)
  R:      R = E8 * broadcast(Q)  (ch 0-5 DVE, ch 6-7 Pool)
  phase3: agg^T[c][d, i] += znat[c][j,:]^T @ R[c]  (fp16 PE, PSUM acc)
  norm:   z = l2norm(z + agg); re-quantize z to fp8 fold layout
  ship:   one merged AllGather (fp16 znat rows + bitcast fp8 zT fold)
          via internal shared DRAM.
Final: out = concat_c(z) @ W_o + bias.
"""

import numpy as np
from contextlib import ExitStack

from concourse import bacc, bass, tile, mybir
from concourse.bass_utils import run_bass_kernel_spmd
from concourse import dve_ops as _dvo
from concourse.dve_spec import Spec, Src0, Src1, C0, C1, AluOp, Bin
from concourse.dve_spec import lower as _dve_lower
from concourse.dve_ops import DveOp, DveOpSpec


def _ref_qrecip(in0, in1, c0, c1, c2):
    x = np.asarray(in0, dtype=np.float32)
    not_x = (~x.view(np.int32)).view(np.float32)
    y0 = not_x * np.float32(c0)
    y1 = y0 * (np.float32(c1) - x * y0)
    return y1 * np.asarray(in1, dtype=np.float32)


def _make_qrecip():
    # Q = mask * approx(1/S): BITWISE_NOT exponent-flip seed + one
    # Newton-Raphson pass (~0.4% rel err, plenty for fp16 weights),
    # fused with the mask multiply. 6 ALU stages.
    not_x = Bin(AluOp.BITWISE_NOT, Src0, Src0)
    y0 = not_x * C0
    y1 = y0 * (C1 - Src0 * y0)
    spec = Spec(body=y1 * Src1, reference=_ref_qrecip)
    name = "QRECIP_ANT"
    opcode = _dvo._CUSTOM_DVE_ROW_BASE + len(_dvo.OPS)
    assert opcode < 0x20
    shas = {}
    for ver in ("v3", "v4"):
        s = DveOpSpec(name=name, opcode=opcode, uops=_dve_lower(spec, ver=ver),
                      rd1_en=True)
        shas[ver] = s.sha(ver)
    op = DveOp(name, spec, subdim=False, uops_sha=shas,
               perf_en={"v3": True, "v4": True})
    _dvo.OPS.append(op)
    _dvo._SUB_OPCODE_FOR_NAME[name] = opcode
    _dvo.CUSTOM_DVE_SPECS[name] = spec
    return op


QRECIP = _make_qrecip()
QRECIP_C0 = float(_dvo.RECIP_APPROX_FAST_CONSTS["s0"])
QRECIP_C1 = float(_dvo.RECIP_APPROX_FAST_CONSTS["s1"])

F32 = mybir.dt.float32
F16 = mybir.dt.float16
F8 = mybir.dt.float8e4
DR = mybir.MatmulPerfMode.DoubleRow

N = 4096
C = 8
IN_DIM = 256
D = 64
OUT = 128
ITERS = 4
NCORES = 8
NL = N // NCORES          # 512 local rows
CD = C * D                # 512
NJT = N // 128            # 32 j-tiles
NPAIR = C // 2            # 4 channel-pair tiles
AF = mybir.ActivationFunctionType
RG = [list(range(NCORES))]
PIPE_DEPTH = 2            # phase3 lags the softmax by this many j-tiles
E8W = C * NL              # 4096: fused E/R tile width


def _build_nc():
    nc = bacc.Bacc(
        "TRN2", target_bir_lowering=False, debug=False, num_devices=NCORES
    )
    featT = nc.dram_tensor("featT", [IN_DIM, NL], F16, kind="ExternalInput").ap()
    wall = nc.dram_tensor("wall", [IN_DIM, CD], F16, kind="ExternalInput").ap()
    bflat = nc.dram_tensor("bflat", [128, NPAIR], F32, kind="ExternalInput").ap()
    maskT = nc.dram_tensor("maskT", [N, NL], F16, kind="ExternalInput").ap()
    wo = nc.dram_tensor("wo", [CD, OUT], F16, kind="ExternalInput").ap()
    biasd = nc.dram_tensor("biasd", [1, OUT], F16, kind="ExternalInput").ap()
    ident = nc.dram_tensor("ident", [128, 128], F16, kind="ExternalInput").ap()
    blkd = nc.dram_tensor("blkd", [128, NPAIR * 8], F16, kind="ExternalInput").ap()
    seld = nc.dram_tensor("seld", [8, NPAIR * 128], F16, kind="ExternalInput").ap()
    onesd = nc.dram_tensor("onesd", [1, 128], F16, kind="ExternalInput").ap()
    outd = nc.dram_tensor("outd", [NL, OUT], F32, kind="ExternalOutput").ap()

    with tile.TileContext(nc) as tc:
        _body(nc, tc, featT, wall, bflat, maskT, wo, biasd, ident, blkd, seld,
              onesd, outd)
    nc.compile()
    return nc


def _body(nc, tc, featT, wall, bflat, maskT, wo, biasd, ident, blkd, seld,
          onesd, outd):
    ctx = ExitStack()
    const = ctx.enter_context(tc.tile_pool(name="const", bufs=1))
    big = ctx.enter_context(tc.tile_pool(name="big", bufs=1))
    work = ctx.enter_context(tc.tile_pool(name="work", bufs=1))
    psum = ctx.enter_context(tc.tile_pool(name="psum", bufs=1, space="PSUM"))
    dram = ctx.enter_context(tc.tile_pool(name="dram", bufs=1, space="DRAM"))

    def loadc(dr_ap, shape, name):
        dst = const.tile(shape, F16, tag=name, bufs=1, name=name)
        nc.sync.dma_start(out=dst, in_=dr_ap)
        return dst

    # ---- constants / weights (fp16 already on host) ----
    ident16 = loadc(ident, [128, 128], "ident16")
    blkd16 = loadc(blkd, [128, NPAIR * 8], "blkd16")
    sel16 = loadc(seld, [8, NPAIR * 128], "sel16")
    ones16 = loadc(onesd, [1, 128], "ones16")
    bT32 = const.tile([128, NPAIR], F32, tag="bT32", bufs=1, name="bT32")
    nc.sync.dma_start(out=bT32, in_=bflat)
    bias16 = loadc(biasd, [1, OUT], "bias16")
    zeros16 = const.tile([1, NL], F16, tag="zeros16", bufs=1, name="zeros16")
    nc.vector.memset(zeros16, 0.0)

    featT16 = const.tile([128, 2 * NL], F16, tag="featT16", bufs=1, name="featT16")
    nc.sync.dma_start(
        out=featT16.rearrange("p (k i) -> p k i", k=2),
        in_=featT.rearrange("(k p) i -> p k i", p=128))
    w016 = const.tile([128, 2 * CD], F16, tag="w016", bufs=1, name="w016")
    nc.sync.dma_start(
        out=w016.rearrange("p (k i) -> p k i", k=2),
        in_=wall.rearrange("(k p) i -> p k i", p=128))
    wo16 = const.tile([128, 4 * OUT], F16, tag="wo16", bufs=1, name="wo16")
    nc.sync.dma_start(
        out=wo16.rearrange("p (k i) -> p k i", k=4),
        in_=wo.rearrange("(k p) i -> p k i", p=128))

    # ---- resident mask (fp16): mask16[:, jt*512 + i] = adj[i_global, j] ----
    # per-jt DMAs on the gpsimd queue so they don't block the ship DMAs
    mask16 = big.tile([128, NJT * NL], F16, tag="mask16", bufs=1, name="mask16")
    for jt in range(NJT):
        nc.gpsimd.dma_start(
            out=mask16[:, jt * NL:(jt + 1) * NL],
            in_=maskT[jt * 128:(jt + 1) * 128, :])

    # ---- skew-absorbing barrier: a tiny AllGather so the first real
    # collective doesn't pay the cross-core launch skew ----
    sk_in = dram.tile([1, 64], F16, tag="skin", bufs=1, name="sk_in")
    sk_src = const.tile([1, 64], F16, tag="sksrc", bufs=1, name="sk_src")
    nc.vector.memset(sk_src, 0.0)
    nc.sync.dma_start(out=sk_in, in_=sk_src)
    sk_out = dram.tile([NCORES, 64], F16, tag="skout", bufs=1,
                       addr_space="Shared", name="sk_out")
    nc.gpsimd.collective_compute(
        "AllGather", mybir.AluOpType.bypass, replica_groups=RG,
        ins=[sk_in.opt()], outs=[sk_out.opt()])

    # ---- resident full z: fp8 folded zT (phase1) + fp16 natural (phase3) ----
    # zT8f: channel c at partitions [(c%2)*64, +32), cols
    #   (c//2)*8*1024 + r*1024 + s*512 + i  (r=rank, s=fold slot: d=s*32+p)
    zT8f = big.tile([128, 4 * 8 * 1024], F8, tag="zT8f", bufs=1, name="zT8f")
    znat16 = big.tile([128, NJT * CD], F16, tag="znat16", bufs=1, name="znat16")

    def normalize_and_rows(zpre, it, want_nat=True):
        """zpre: 4 SBUF fp16 tiles [128, NL] (z_T rows layout, pre-norm).
        Returns (zrows, natrows): l2-normalized rows in both layouts."""
        nrm = psum.tile([8, NL], F32, tag="L", bufs=2, name=f"nrm_{it}")
        for t in range(NPAIR):
            sq = work.tile([128, NL], F16, tag="sq", bufs=2, name=f"sq_{it}_{t}")
            nc.vector.tensor_mul(out=sq, in0=zpre[t], in1=zpre[t])
            nc.tensor.matmul(out=nrm, lhsT=blkd16[:, t * 8:(t + 1) * 8], rhs=sq,
                             start=(t == 0), stop=(t == NPAIR - 1))
        rsq = work.tile([8, NL], F16, tag="rsq", bufs=2, name=f"rsq_{it}")
        # rsqrt straight from PSUM (sumsq of this data is bounded >> 1e-12,
        # so the reference's clamp is a numeric no-op)
        nc.scalar.activation(out=rsq, in_=nrm, func=AF.Abs_reciprocal_sqrt)
        zrows = []
        for t in range(NPAIR):
            bc = psum.tile([128, NL], F32, tag="L", bufs=2, name=f"bc_{it}_{t}")
            nc.tensor.matmul(out=bc, lhsT=sel16[:, t * 128:(t + 1) * 128],
                             rhs=rsq, start=True, stop=True)
            zr = work.tile([128, NL], F16, tag="zrows", bufs=8,
                           name=f"zrows_{it}_{t}")
            nc.vector.tensor_mul(out=zr, in0=zpre[t], in1=bc)
            zrows.append(zr)
        if not want_nat:
            return zrows, None
        natrows = [work.tile([128, CD], F16, tag="natrows", bufs=4,
                             name=f"natr_{it}_{ib}") for ib in range(4)]
        for t in range(NPAIR):
            for ib in range(4):
                tp = psum.tile([128, 128], F16, tag="L", bufs=2,
                               name=f"tp_{it}_{t}_{ib}")
                nc.tensor.transpose(out=tp,
                                    in_=zrows[t][:, ib * 128:(ib + 1) * 128],
                                    identity=ident16)
                nc.vector.tensor_copy(
                    out=natrows[ib][:, t * 128:(t + 1) * 128], in_=tp)
        return zrows, natrows

    AGR = 768  # rows per rank in the merged AllGather buffer

    def ship_all(zrows, natrows, it):
        """One merged AllGather: rows 0-511 nat fp16, rows 512-767 hold the
        fp8 folded zT (bitcast into the fp16 buffer). Returns the local
        folded rhs tile zfold8: channel c at partitions [(c%2)*64, +32),
        cols (c//2)*1024 + s*512 + i, where z[c, i, d] sits at fold
        partition p=d%32, slot s=d//32."""
        ag_in = dram.tile([AGR, CD], F16, tag="agin", bufs=2,
                          name=f"agin_{it}")
        ag8i = ag_in.bitcast(F8)          # [768, 1024] byte view
        for ib in range(4):
            nc.sync.dma_start(out=ag_in[ib * 128:(ib + 1) * 128, :],
                              in_=natrows[ib])
        for t in range(NPAIR):
            z8 = work.tile([128, NL], F8, tag="z8", bufs=4,
                           name=f"z8_{it}_{t}")
            # fp16 -> fp8 copy on ACT (Copy is in the exp table set)
            nc.scalar.activation(out=z8, in_=zrows[t], func=AF.Copy)
            for h in range(2):
                c = 2 * t + h
                b = c % 2
                cc = c // 2
                # fold row fr=b*32+p, byte j=cc*1024+s*512+i sits at
                # fp16-row 512+fr*4+j//1024, byte-col j%1024
                nc.sync.dma_start(
                    out=ag8i[512 + b * 128:512 + (b + 1) * 128, :]
                        .rearrange("(p four) i -> p four i", four=4)
                        [:, cc, :]
                        .rearrange("p (s i) -> s p i", s=2),
                    in_=z8[h * 64:(h + 1) * 64, :])
        # local folded rhs for phase1 (round-trip through ag_in)
        zfold8 = work.tile([128, 4 * 1024], F8, tag="zfold8", bufs=2,
                           name=f"zfold8_{it}")
        nc.sync.dma_start(
            out=zfold8[0:32, :],
            in_=ag8i[512:640, :].rearrange("(p four) i -> p (four i)", four=4))
        nc.sync.dma_start(
            out=zfold8[64:96, :],
            in_=ag8i[640:768, :].rearrange("(p four) i -> p (four i)", four=4))
        ag_out = dram.tile([NCORES * AGR, CD], F16, tag="agout", bufs=2,
                           addr_space="Shared", name=f"agout_{it}")
        nc.gpsimd.collective_compute(
            "AllGather", mybir.AluOpType.bypass, replica_groups=RG,
            ins=[ag_in.opt()], outs=[ag_out.opt()])
        ag8o = ag_out.bitcast(F8)
        for r in range(NCORES):
            # nat readback: one DMA per rank
            nc.sync.dma_start(
                out=znat16[:, r * 4 * CD:(r + 1) * 4 * CD]
                    .rearrange("p (pb d) -> p pb d", pb=4),
                in_=ag_out[r * AGR:r * AGR + 512, :]
                    .rearrange("(pb p) d -> p pb d", pb=4))
            # fold readback on the gpsimd queue (phase1-critical)
            for b in range(2):
                nc.gpsimd.dma_start(
                    out=zT8f[b * 64:b * 64 + 32, :]
                        .rearrange("p (cc rr f) -> p cc rr f", cc=4, rr=8)
                        [:, :, r, :],
                    in_=ag8o[r * AGR + 512 + b * 128:
                             r * AGR + 512 + (b + 1) * 128, :]
                        .rearrange("(p four) i -> p four i", four=4))
        return zfold8

    # ===== phase 0: z0 = l2norm(features @ W + b), built in z_T layout =====
    zpre0 = []
    for t in range(NPAIR):
        zp = psum.tile([128, NL], F32, tag="L", bufs=2, name=f"zp_{t}")
        for kt in range(2):
            nc.tensor.matmul(
                out=zp,
                lhsT=w016[:, kt * CD + t * 128:kt * CD + (t + 1) * 128],
                rhs=featT16[:, kt * NL:(kt + 1) * NL],
                start=(kt == 0), stop=(kt == 1))
        zt = work.tile([128, NL], F16, tag="zpre0", bufs=5, name=f"zpre0_{t}")
        nc.scalar.activation(out=zt, in_=zp, func=AF.Identity,
                             bias=bT32[:, t:t + 1])
        zpre0.append(zt)
    zrows, natrows = normalize_and_rows(zpre0, it=-1)
    zfold8 = ship_all(zrows, natrows, it=-1)

    # ================= routing iterations =================
    for it in range(ITERS):
        agg = [psum.tile([128, NL], F32, tag="agg", bufs=4, name=f"agg_{it}_{t}")
               for t in range(NPAIR)]
        for t in range(NPAIR):
            # zero-fill the whole bank once so both col-tiled halves can
            # accumulate with start=False (start clears the full bank)
            nc.tensor.matmul(out=agg[t], lhsT=ones16, rhs=zeros16,
                             start=True, stop=False)
        pending = []
        for jt in range(NJT):
            E8 = work.tile([128, E8W], F16, tag="E", bufs=3,
                           name=f"E8_{it}_{jt}")
            for t in range(NPAIR):
                L2 = psum.tile([128, 2 * NL], F32, tag="L", bufs=2,
                               name=f"L2_{it}_{jt}_{t}")
                for h in range(2):
                    c = 2 * t + h
                    cb = (c % 2) * 64       # partition base
                    cc = c // 2             # column block
                    nc.tensor.matmul(
                        out=L2[:, h * NL:(h + 1) * NL],
                        lhsT=zT8f[cb:cb + 32,
                                  cc * 8192 + 0:cc * 8192 + 8192]
                            .rearrange("p (rr s i) -> p rr s i", rr=8, s=2)
                            [:, jt // 4, :, (jt % 4) * 128:(jt % 4 + 1) * 128],
                        rhs=zfold8[cb:cb + 32, cc * 1024:(cc + 1) * 1024]
                            .rearrange("p (s i) -> p s i", s=2),
                        start=True, stop=True, perf_mode=DR,
                        tile_position=(cb, 0))
                nc.scalar.activation(
                    out=E8[:, t * 2 * NL:(t + 1) * 2 * NL], in_=L2,
                    func=AF.Exp)
            # channel-softmax denominator: tree sum, all on DVE (running
            # it on Pool concurrently contends for the same SBUF tiles and
            # slows both engines down)
            u = work.tile([128, 2 * NL], F16, tag="s2", bufs=6,
                          name=f"u_{it}_{jt}")
            nc.vector.tensor_add(out=u, in0=E8[:, 0:1024], in1=E8[:, 1024:2048])
            v = work.tile([128, 2 * NL], F16, tag="s2", bufs=6,
                          name=f"v_{it}_{jt}")
            nc.vector.tensor_add(out=v, in0=E8[:, 2048:3072],
                                 in1=E8[:, 3072:4096])
            w = work.tile([128, 2 * NL], F16, tag="s2", bufs=6,
                          name=f"w_{it}_{jt}")
            nc.vector.tensor_add(out=w, in0=u, in1=v)
            S16 = work.tile([128, NL], F16, tag="S16", bufs=4,
                            name=f"S16_{it}_{jt}")
            nc.vector.tensor_add(out=S16, in0=w[:, 0:NL], in1=w[:, NL:])
            # Q = mask * 1/S in one fused custom-DVE op
            Q = work.tile([128, NL], F16, tag="Q", bufs=4, name=f"Q_{it}_{jt}")
            nc.vector._custom_dve(
                QRECIP, out=Q, in0=S16,
                in1=mask16[:, jt * NL:(jt + 1) * NL],
                s0=QRECIP_C0, s1=QRECIP_C1)
            # R8 = E8 * broadcast(Q) over the 8 channel blocks, one DVE op
            R8 = work.tile([128, E8W], F16, tag="R", bufs=PIPE_DEPTH + 1,
                           name=f"R8_{it}_{jt}")
            nc.vector.tensor_mul(
                out=R8.rearrange("p (c i) -> p c i", c=C),
                in0=E8.rearrange("p (c i) -> p c i", c=C),
                in1=Q.unsqueeze(1).broadcast_to([128, C, NL]))
            pending.append((jt, R8))
            if len(pending) > PIPE_DEPTH:
                pjt, pR8 = pending.pop(0)
                for c in range(C):
                    t, h = c // 2, c % 2
                    nc.tensor.matmul(
                        out=agg[t][h * 64:(h + 1) * 64, :],
                        lhsT=znat16[:, pjt * CD + c * 64:pjt * CD + (c + 1) * 64],
                        rhs=pR8[:, c * NL:(c + 1) * NL],
                        start=False, stop=False,
                        tile_position=(0, h * 64))
        for pjt, pR8 in pending:
            for c in range(C):
                t, h = c // 2, c % 2
                nc.tensor.matmul(
                    out=agg[t][h * 64:(h + 1) * 64, :],
                    lhsT=znat16[:, pjt * CD + c * 64:pjt * CD + (c + 1) * 64],
                    rhs=pR8[:, c * NL:(c + 1) * NL],
                    start=False, stop=False,
                    tile_position=(0, h * 64))
        for t in range(NPAIR):
            # N=1 dummy stop: closes the sim accumulation group, no-op on HW
            nc.tensor.matmul(out=agg[t][:, 0:1], lhsT=ones16,
                             rhs=zeros16[:, 0:1], start=False, stop=True)
        # residual + renorm
        zpre = []
        for t in range(NPAIR):
            zq = work.tile([128, NL], F16, tag="zpre0", bufs=5,
                           name=f"zpre_{it}_{t}")
            nc.vector.tensor_add(out=zq, in0=zrows[t], in1=agg[t])
            zpre.append(zq)
        zrows, natrows = normalize_and_rows(zpre, it=it,
                                            want_nat=(it < ITERS - 1))
        if it < ITERS - 1:
            zfold8 = ship_all(zrows, natrows, it=it)

    # ================= output: h @ W_o + bias =================
    for ib in range(4):
        op = psum.tile([128, OUT], F32, tag="L", bufs=2, name=f"op_{ib}")
        for kt in range(4):
            nc.tensor.matmul(out=op,
                             lhsT=zrows[kt][:, ib * 128:(ib + 1) * 128],
                             rhs=wo16[:, kt * OUT:(kt + 1) * OUT],
                             start=(kt == 0), stop=False)
        nc.tensor.matmul(out=op, lhsT=ones16, rhs=bias16, start=False, stop=True)
        ot = work.tile([128, OUT], F32, tag="ot", bufs=2, name=f"ot_{ib}")
        nc.vector.tensor_copy(out=ot, in_=op)
        nc.sync.dma_start(out=outd[ib * 128:(ib + 1) * 128, :], in_=ot)

    ctx.close()


def _make_in_maps(features, adj, W, b, W_o, bias):
    import ml_dtypes
    features = np.asarray(features, dtype=np.float32)
    adj = np.asarray(adj, dtype=np.float32)
    W = np.asarray(W, dtype=np.float32)
    b = np.asarray(b, dtype=np.float32)
    W_o = np.asarray(W_o, dtype=np.float32)
    bias = np.asarray(bias, dtype=np.float32)

    f16 = np.float16
    f8 = ml_dtypes.float8_e4m3
    wall = np.ascontiguousarray(
        W.transpose(1, 0, 2).reshape(IN_DIM, CD)).astype(f16)
    bflat = np.ascontiguousarray(b.reshape(1, CD).reshape(NPAIR, 128).T).astype(np.float32)
    ident = np.eye(128, dtype=f16)
    blkd = np.zeros((128, NPAIR * 8), dtype=f16)
    seld = np.zeros((8, NPAIR * 128), dtype=f16)
    for t in range(NPAIR):
        for h in range(2):
            c = 2 * t + h
            blkd[h * 64:(h + 1) * 64, t * 8 + c] = 1.0
            seld[c, t * 128 + h * 64:t * 128 + (h + 1) * 64] = 1.0
    onesd = np.ones((1, 128), dtype=f16)
    wo16 = W_o.astype(f16)
    bias16 = bias.reshape(1, OUT).astype(f16)

    in_maps = []
    for r in range(NCORES):
        rows = slice(r * NL, (r + 1) * NL)
        in_maps.append({
            "featT": np.ascontiguousarray(features[rows].T).astype(f16),
            "wall": wall,
            "bflat": bflat,
            "maskT": np.ascontiguousarray(adj[rows].T).astype(f16),
            "wo": wo16,
            "biasd": bias16,
            "ident": ident,
            "blkd": blkd,
            "seld": seld,
            "onesd": onesd,
        })
    return in_maps


_NC_CACHE = []


def _get_nc():
    if not _NC_CACHE:
        _NC_CACHE.append(_build_nc())
    return _NC_CACHE[0]


def run(inputs, trace=False, **kwargs):
    nc = _get_nc()
    in_maps = _make_in_maps(**inputs)
    res = run_bass_kernel_spmd(nc, in_maps, core_ids=list(range(NCORES)),
                               trace=trace, **kwargs)
    out = np.concatenate([res.results[r]["outd"] for r in range(NCORES)],
                         axis=0).astype(np.float32)
    return out, res


def kernel(features, adj, W, b, W_o, bias):
    out, _ = run(dict(features=features, adj=adj, W=W, b=b, W_o=W_o, bias=bias))
    return out


# revision 20
# speedup vs baseline: 1.2970x; 1.0017x over previous
"""Disen-GCN (8-channel routing attention GNN) on 8 TRN2 NeuronCores.

Row-parallel sharding: core r owns node rows [r*512, (r+1)*512).
Per routing iteration:
  phase1: L[c][j, i] = z[c,j] . z[c,i]  fp8e4 DoubleRow matmul (K=64
          folded 2-per-partition onto 32 partitions; 256 PE cyc/instr)
  exp:    E8[:, c*512+i] = exp(L)       (ACT, PSUM->SBUF fp16)
  smax:   S = sum_c E8 (v-add on Pool); Q = mask * 1/S (DVE)
  R:      R = E8 * broadcast(Q)  (ch 0-5 DVE, ch 6-7 Pool)
  phase3: agg^T[c][d, i] += znat[c][j,:]^T @ R[c]  (fp16 PE, PSUM acc)
  norm:   z = l2norm(z + agg); re-quantize z to fp8 fold layout
  ship:   one merged AllGather (fp16 znat rows + bitcast fp8 zT fold)
          via internal shared DRAM.
Final: out = concat_c(z) @ W_o + bias.
"""

import numpy as np
from contextlib import ExitStack

from concourse import bacc, bass, tile, mybir
from concourse.bass_utils import run_bass_kernel_spmd
from concourse import dve_ops as _dvo
from concourse.dve_spec import Spec, Src0, Src1, C0, C1, AluOp, Bin
from concourse.dve_spec import lower as _dve_lower
from concourse.dve_ops import DveOp, DveOpSpec


def _ref_qrecip(in0, in1, c0, c1, c2):
    x = np.asarray(in0, dtype=np.float32)
    not_x = (~x.view(np.int32)).view(np.float32)
    y0 = not_x * np.float32(c0)
    y1 = y0 * (np.float32(c1) - x * y0)
    return y1 * np.asarray(in1, dtype=np.float32)


def _make_qrecip():
    # Q = mask * approx(1/S): BITWISE_NOT exponent-flip seed + one
    # Newton-Raphson pass (~0.4% rel err, plenty for fp16 weights),
    # fused with the mask multiply. 6 ALU stages.
    not_x = Bin(AluOp.BITWISE_NOT, Src0, Src0)
    y0 = not_x * C0
    y1 = y0 * (C1 - Src0 * y0)
    spec = Spec(body=y1 * Src1, reference=_ref_qrecip)
    name = "QRECIP_ANT"
    opcode = _dvo._CUSTOM_DVE_ROW_BASE + len(_dvo.OPS)
    assert opcode < 0x20
    shas = {}
    for ver in ("v3", "v4"):
        s = DveOpSpec(name=name, opcode=opcode, uops=_dve_lower(spec, ver=ver),
                      rd1_en=True)
        shas[ver] = s.sha(ver)
    op = DveOp(name, spec, subdim=False, uops_sha=shas,
               perf_en={"v3": True, "v4": True})
    _dvo.OPS.append(op)
    _dvo._SUB_OPCODE_FOR_NAME[name] = opcode
    _dvo.CUSTOM_DVE_SPECS[name] = spec
    return op


QRECIP = _make_qrecip()
QRECIP_C0 = float(_dvo.RECIP_APPROX_FAST_CONSTS["s0"])
QRECIP_C1 = float(_dvo.RECIP_APPROX_FAST_CONSTS["s1"])

F32 = mybir.dt.float32
F16 = mybir.dt.float16
F8 = mybir.dt.float8e4
DR = mybir.MatmulPerfMode.DoubleRow

N = 4096
C = 8
IN_DIM = 256
D = 64
OUT = 128
ITERS = 4
NCORES = 8
NL = N // NCORES          # 512 local rows
CD = C * D                # 512
NJT = N // 128            # 32 j-tiles
NPAIR = C // 2            # 4 channel-pair tiles
AF = mybir.ActivationFunctionType
RG = [list(range(NCORES))]
PIPE_DEPTH = 2            # phase3 lags the softmax by this many j-tiles
E8W = C * NL              # 4096: fused E/R tile width


def _build_nc():
    nc = bacc.Bacc(
        "TRN2", target_bir_lowering=False, debug=False, num_devices=NCORES
    )
    featT = nc.dram_tensor("featT", [IN_DIM, NL], F16, kind="ExternalInput").ap()
    wall = nc.dram_tensor("wall", [IN_DIM, CD], F16, kind="ExternalInput").ap()
    bflat = nc.dram_tensor("bflat", [128, NPAIR], F32, kind="ExternalInput").ap()
    maskT = nc.dram_tensor("maskT", [N, NL], F16, kind="ExternalInput").ap()
    wo = nc.dram_tensor("wo", [CD, OUT], F16, kind="ExternalInput").ap()
    biasd = nc.dram_tensor("biasd", [1, OUT], F16, kind="ExternalInput").ap()
    ident = nc.dram_tensor("ident", [128, 128], F16, kind="ExternalInput").ap()
    blkd = nc.dram_tensor("blkd", [128, NPAIR * 8], F16, kind="ExternalInput").ap()
    seld = nc.dram_tensor("seld", [8, NPAIR * 128], F16, kind="ExternalInput").ap()
    onesd = nc.dram_tensor("onesd", [1, 128], F16, kind="ExternalInput").ap()
    outd = nc.dram_tensor("outd", [NL, OUT], F32, kind="ExternalOutput").ap()

    with tile.TileContext(nc) as tc:
        _body(nc, tc, featT, wall, bflat, maskT, wo, biasd, ident, blkd, seld,
              onesd, outd)
    nc.compile()
    return nc


def _body(nc, tc, featT, wall, bflat, maskT, wo, biasd, ident, blkd, seld,
          onesd, outd):
    ctx = ExitStack()
    const = ctx.enter_context(tc.tile_pool(name="const", bufs=1))
    big = ctx.enter_context(tc.tile_pool(name="big", bufs=1))
    work = ctx.enter_context(tc.tile_pool(name="work", bufs=1))
    psum = ctx.enter_context(tc.tile_pool(name="psum", bufs=1, space="PSUM"))
    dram = ctx.enter_context(tc.tile_pool(name="dram", bufs=1, space="DRAM"))

    def loadc(dr_ap, shape, name):
        dst = const.tile(shape, F16, tag=name, bufs=1, name=name)
        nc.sync.dma_start(out=dst, in_=dr_ap)
        return dst

    # ---- constants / weights (fp16 already on host) ----
    ident16 = loadc(ident, [128, 128], "ident16")
    blkd16 = loadc(blkd, [128, NPAIR * 8], "blkd16")
    sel16 = loadc(seld, [8, NPAIR * 128], "sel16")
    ones16 = loadc(onesd, [1, 128], "ones16")
    bT32 = const.tile([128, NPAIR], F32, tag="bT32", bufs=1, name="bT32")
    nc.sync.dma_start(out=bT32, in_=bflat)
    bias16 = loadc(biasd, [1, OUT], "bias16")
    zeros16 = const.tile([1, NL], F16, tag="zeros16", bufs=1, name="zeros16")
    nc.vector.memset(zeros16, 0.0)

    featT16 = const.tile([128, 2 * NL], F16, tag="featT16", bufs=1, name="featT16")
    nc.sync.dma_start(
        out=featT16.rearrange("p (k i) -> p k i", k=2),
        in_=featT.rearrange("(k p) i -> p k i", p=128))
    w016 = const.tile([128, 2 * CD], F16, tag="w016", bufs=1, name="w016")
    nc.sync.dma_start(
        out=w016.rearrange("p (k i) -> p k i", k=2),
        in_=wall.rearrange("(k p) i -> p k i", p=128))
    wo16 = const.tile([128, 4 * OUT], F16, tag="wo16", bufs=1, name="wo16")
    nc.sync.dma_start(
        out=wo16.rearrange("p (k i) -> p k i", k=4),
        in_=wo.rearrange("(k p) i -> p k i", p=128))

    # ---- resident mask (fp16): mask16[:, jt*512 + i] = adj[i_global, j] ----
    # per-jt DMAs on the gpsimd queue so they don't block the ship DMAs
    mask16 = big.tile([128, NJT * NL], F16, tag="mask16", bufs=1, name="mask16")
    for jt in range(NJT):
        nc.gpsimd.dma_start(
            out=mask16[:, jt * NL:(jt + 1) * NL],
            in_=maskT[jt * 128:(jt + 1) * 128, :])

    # ---- skew-absorbing barrier: a tiny AllGather so the first real
    # collective doesn't pay the cross-core launch skew ----
    sk_in = dram.tile([1, 64], F16, tag="skin", bufs=1, name="sk_in")
    sk_src = const.tile([1, 64], F16, tag="sksrc", bufs=1, name="sk_src")
    nc.vector.memset(sk_src, 0.0)
    nc.sync.dma_start(out=sk_in, in_=sk_src)
    sk_out = dram.tile([NCORES, 64], F16, tag="skout", bufs=1,
                       addr_space="Shared", name="sk_out")
    nc.gpsimd.collective_compute(
        "AllGather", mybir.AluOpType.bypass, replica_groups=RG,
        ins=[sk_in.opt()], outs=[sk_out.opt()])

    # ---- resident full z: fp8 folded zT (phase1) + fp16 natural (phase3) ----
    # zT8f: channel c at partitions [(c%2)*64, +32), cols
    #   (c//2)*8*1024 + r*1024 + s*512 + i  (r=rank, s=fold slot: d=s*32+p)
    zT8f = big.tile([128, 4 * 8 * 1024], F8, tag="zT8f", bufs=1, name="zT8f")
    znat16 = big.tile([128, NJT * CD], F16, tag="znat16", bufs=1, name="znat16")

    def normalize_and_rows(zpre, it, want_nat=True):
        """zpre: 4 SBUF fp16 tiles [128, NL] (z_T rows layout, pre-norm).
        Returns (zrows, natrows): l2-normalized rows in both layouts."""
        nrm = psum.tile([8, NL], F32, tag="L", bufs=2, name=f"nrm_{it}")
        for t in range(NPAIR):
            sq = work.tile([128, NL], F16, tag="sq", bufs=2, name=f"sq_{it}_{t}")
            nc.vector.tensor_mul(out=sq, in0=zpre[t], in1=zpre[t])
            nc.tensor.matmul(out=nrm, lhsT=blkd16[:, t * 8:(t + 1) * 8], rhs=sq,
                             start=(t == 0), stop=(t == NPAIR - 1))
        rsq = work.tile([8, NL], F16, tag="rsq", bufs=2, name=f"rsq_{it}")
        # rsqrt straight from PSUM (sumsq of this data is bounded >> 1e-12,
        # so the reference's clamp is a numeric no-op)
        nc.scalar.activation(out=rsq, in_=nrm, func=AF.Abs_reciprocal_sqrt)
        zrows = []
        for t in range(NPAIR):
            bc = psum.tile([128, NL], F32, tag="L", bufs=2, name=f"bc_{it}_{t}")
            nc.tensor.matmul(out=bc, lhsT=sel16[:, t * 128:(t + 1) * 128],
                             rhs=rsq, start=True, stop=True)
            zr = work.tile([128, NL], F16, tag="zrows", bufs=8,
                           name=f"zrows_{it}_{t}")
            nc.vector.tensor_mul(out=zr, in0=zpre[t], in1=bc)
            zrows.append(zr)
        if not want_nat:
            return zrows, None
        natrows = [work.tile([128, CD], F16, tag="natrows", bufs=4,
                             name=f"natr_{it}_{ib}") for ib in range(4)]
        for t in range(NPAIR):
            for ib in range(4):
                tp = psum.tile([128, 128], F16, tag="L", bufs=2,
                               name=f"tp_{it}_{t}_{ib}")
                nc.tensor.transpose(out=tp,
                                    in_=zrows[t][:, ib * 128:(ib + 1) * 128],
                                    identity=ident16)
                nc.vector.tensor_copy(
                    out=natrows[ib][:, t * 128:(t + 1) * 128], in_=tp)
        return zrows, natrows

    AGR = 768  # rows per rank in the merged AllGather buffer

    def ship_all(zrows, natrows, it):
        """One merged AllGather: rows 0-511 nat fp16, rows 512-767 hold the
        fp8 folded zT (bitcast into the fp16 buffer). Returns the local
        folded rhs tile zfold8: channel c at partitions [(c%2)*64, +32),
        cols (c//2)*1024 + s*512 + i, where z[c, i, d] sits at fold
        partition p=d%32, slot s=d//32."""
        ag_in = dram.tile([AGR, CD], F16, tag="agin", bufs=2,
                          name=f"agin_{it}")
        ag8i = ag_in.bitcast(F8)          # [768, 1024] byte view
        for ib in range(4):
            nc.sync.dma_start(out=ag_in[ib * 128:(ib + 1) * 128, :],
                              in_=natrows[ib])
        for t in range(NPAIR):
            z8 = work.tile([128, NL], F8, tag="z8", bufs=4,
                           name=f"z8_{it}_{t}")
            # fp16 -> fp8 copy on ACT (Copy is in the exp table set)
            nc.scalar.activation(out=z8, in_=zrows[t], func=AF.Copy)
            for h in range(2):
                c = 2 * t + h
                b = c % 2
                cc = c // 2
                # fold row fr=b*32+p, byte j=cc*1024+s*512+i sits at
                # fp16-row 512+fr*4+j//1024, byte-col j%1024
                nc.sync.dma_start(
                    out=ag8i[512 + b * 128:512 + (b + 1) * 128, :]
                        .rearrange("(p four) i -> p four i", four=4)
                        [:, cc, :]
                        .rearrange("p (s i) -> s p i", s=2),
                    in_=z8[h * 64:(h + 1) * 64, :])
        # local folded rhs for phase1 (round-trip through ag_in)
        zfold8 = work.tile([128, 4 * 1024], F8, tag="zfold8", bufs=2,
                           name=f"zfold8_{it}")
        nc.sync.dma_start(
            out=zfold8[0:32, :],
            in_=ag8i[512:640, :].rearrange("(p four) i -> p (four i)", four=4))
        nc.sync.dma_start(
            out=zfold8[64:96, :],
            in_=ag8i[640:768, :].rearrange("(p four) i -> p (four i)", four=4))
        ag_out = dram.tile([NCORES * AGR, CD], F16, tag="agout", bufs=2,
                           addr_space="Shared", name=f"agout_{it}")
        nc.gpsimd.collective_compute(
            "AllGather", mybir.AluOpType.bypass, replica_groups=RG,
            ins=[ag_in.opt()], outs=[ag_out.opt()])
        ag8o = ag_out.bitcast(F8)
        for r in range(NCORES):
            # nat readback: one DMA per rank
            nc.sync.dma_start(
                out=znat16[:, r * 4 * CD:(r + 1) * 4 * CD]
                    .rearrange("p (pb d) -> p pb d", pb=4),
                in_=ag_out[r * AGR:r * AGR + 512, :]
                    .rearrange("(pb p) d -> p pb d", pb=4))
            # fold readback on the gpsimd queue (phase1-critical)
            for b in range(2):
                nc.gpsimd.dma_start(
                    out=zT8f[b * 64:b * 64 + 32, :]
                        .rearrange("p (cc rr f) -> p cc rr f", cc=4, rr=8)
                        [:, :, r, :],
                    in_=ag8o[r * AGR + 512 + b * 128:
                             r * AGR + 512 + (b + 1) * 128, :]
                        .rearrange("(p four) i -> p four i", four=4))
        return zfold8

    # ===== phase 0: z0 = l2norm(features @ W + b), built in z_T layout =====
    zpre0 = []
    for t in range(NPAIR):
        zp = psum.tile([128, NL], F32, tag="L", bufs=2, name=f"zp_{t}")
        for kt in range(2):
            nc.tensor.matmul(
                out=zp,
                lhsT=w016[:, kt * CD + t * 128:kt * CD + (t + 1) * 128],
                rhs=featT16[:, kt * NL:(kt + 1) * NL],
                start=(kt == 0), stop=(kt == 1))
        zt = work.tile([128, NL], F16, tag="zpre0", bufs=5, name=f"zpre0_{t}")
        nc.scalar.activation(out=zt, in_=zp, func=AF.Identity,
                             bias=bT32[:, t:t + 1])
        zpre0.append(zt)
    zrows, natrows = normalize_and_rows(zpre0, it=-1)
    zfold8 = ship_all(zrows, natrows, it=-1)

    # ================= routing iterations =================
    for it in range(ITERS):
        agg = [psum.tile([128, NL], F32, tag="agg", bufs=4, name=f"agg_{it}_{t}")
               for t in range(NPAIR)]
        for t in range(NPAIR):
            # zero-fill the whole bank once so both col-tiled halves can
            # accumulate with start=False (start clears the full bank)
            nc.tensor.matmul(out=agg[t], lhsT=ones16, rhs=zeros16,
                             start=True, stop=False)
        pending = []
        for jt in range(NJT):
            E8 = work.tile([128, E8W], F16, tag="E", bufs=3,
                           name=f"E8_{it}_{jt}")
            for t in range(NPAIR):
                L2 = psum.tile([128, 2 * NL], F32, tag="L", bufs=2,
                               name=f"L2_{it}_{jt}_{t}")
                for h in range(2):
                    c = 2 * t + h
                    cb = (c % 2) * 64       # partition base
                    cc = c // 2             # column block
                    nc.tensor.matmul(
                        out=L2[:, h * NL:(h + 1) * NL],
                        lhsT=zT8f[cb:cb + 32,
                                  cc * 8192 + 0:cc * 8192 + 8192]
                            .rearrange("p (rr s i) -> p rr s i", rr=8, s=2)
                            [:, jt // 4, :, (jt % 4) * 128:(jt % 4 + 1) * 128],
                        rhs=zfold8[cb:cb + 32, cc * 1024:(cc + 1) * 1024]
                            .rearrange("p (s i) -> p s i", s=2),
                        start=True, stop=True, perf_mode=DR,
                        tile_position=(cb, 0))
                nc.scalar.activation(
                    out=E8[:, t * 2 * NL:(t + 1) * 2 * NL], in_=L2,
                    func=AF.Exp)
            # channel-softmax denominator: tree sum, all on DVE (running
            # it on Pool concurrently contends for the same SBUF tiles and
            # slows both engines down)
            u = work.tile([128, 2 * NL], F16, tag="s2", bufs=6,
                          name=f"u_{it}_{jt}")
            nc.vector.tensor_add(out=u, in0=E8[:, 0:1024], in1=E8[:, 1024:2048])
            v = work.tile([128, 2 * NL], F16, tag="s2", bufs=6,
                          name=f"v_{it}_{jt}")
            nc.vector.tensor_add(out=v, in0=E8[:, 2048:3072],
                                 in1=E8[:, 3072:4096])
            w = work.tile([128, 2 * NL], F16, tag="s2", bufs=6,
                          name=f"w_{it}_{jt}")
            nc.vector.tensor_add(out=w, in0=u, in1=v)
            S16 = work.tile([128, NL], F16, tag="S16", bufs=4,
                            name=f"S16_{it}_{jt}")
            nc.vector.tensor_add(out=S16, in0=w[:, 0:NL], in1=w[:, NL:])
            # Q = mask * 1/S in one fused custom-DVE op
            Q = work.tile([128, NL], F16, tag="Q", bufs=4, name=f"Q_{it}_{jt}")
            nc.vector._custom_dve(
                QRECIP, out=Q, in0=S16,
                in1=mask16[:, jt * NL:(jt + 1) * NL],
                s0=QRECIP_C0, s1=QRECIP_C1)
            # R8 = E8 * broadcast(Q) over the 8 channel blocks, one DVE op
            R8 = work.tile([128, E8W], F16, tag="R", bufs=PIPE_DEPTH + 1,
                           name=f"R8_{it}_{jt}")
            nc.vector.tensor_mul(
                out=R8.rearrange("p (c i) -> p c i", c=C),
                in0=E8.rearrange("p (c i) -> p c i", c=C),
                in1=Q.unsqueeze(1).broadcast_to([128, C, NL]))
            pending.append((jt, R8))
            if len(pending) > PIPE_DEPTH:
                pjt, pR8 = pending.pop(0)
                for c in range(C):
                    t, h = c // 2, c % 2
                    nc.tensor.matmul(
                        out=agg[t][h * 64:(h + 1) * 64, :],
                        lhsT=znat16[:, pjt * CD + c * 64:pjt * CD + (c + 1) * 64],
                        rhs=pR8[:, c * NL:(c + 1) * NL],
                        start=False, stop=False,
                        tile_position=(0, h * 64))
        for pjt, pR8 in pending:
            for c in range(C):
                t, h = c // 2, c % 2
                nc.tensor.matmul(
                    out=agg[t][h * 64:(h + 1) * 64, :],
                    lhsT=znat16[:, pjt * CD + c * 64:pjt * CD + (c + 1) * 64],
                    rhs=pR8[:, c * NL:(c + 1) * NL],
                    start=False, stop=False,
                    tile_position=(0, h * 64))
        for t in range(NPAIR):
            # N=1 dummy stop: closes the sim accumulation group, no-op on HW
            nc.tensor.matmul(out=agg[t][:, 0:1], lhsT=ones16,
                             rhs=zeros16[:, 0:1], start=False, stop=True)
        # residual + renorm
        zpre = []
        for t in range(NPAIR):
            zq = work.tile([128, NL], F16, tag="zpre0", bufs=5,
                           name=f"zpre_{it}_{t}")
            nc.vector.tensor_add(out=zq, in0=zrows[t], in1=agg[t])
            zpre.append(zq)
        zrows, natrows = normalize_and_rows(zpre, it=it,
                                            want_nat=(it < ITERS - 1))
        if it < ITERS - 1:
            zfold8 = ship_all(zrows, natrows, it=it)

    # ================= output: h @ W_o + bias =================
    for ib in range(4):
        op = psum.tile([128, OUT], F32, tag="L", bufs=2, name=f"op_{ib}")
        for kt in range(4):
            nc.tensor.matmul(out=op,
                             lhsT=zrows[kt][:, ib * 128:(ib + 1) * 128],
                             rhs=wo16[:, kt * OUT:(kt + 1) * OUT],
                             start=(kt == 0), stop=False)
        nc.tensor.matmul(out=op, lhsT=ones16, rhs=bias16, start=False, stop=True)
        ot = work.tile([128, OUT], F32, tag="ot", bufs=2, name=f"ot_{ib}")
        nc.vector.tensor_copy(out=ot, in_=op)
        nc.sync.dma_start(out=outd[ib * 128:(ib + 1) * 128, :], in_=ot)

    ctx.close()


def _make_in_maps(features, adj, W, b, W_o, bias):
    import ml_dtypes
    features = np.asarray(features, dtype=np.float32)
    adj = np.asarray(adj, dtype=np.float32)
    W = np.asarray(W, dtype=np.float32)
    b = np.asarray(b, dtype=np.float32)
    W_o = np.asarray(W_o, dtype=np.float32)
    bias = np.asarray(bias, dtype=np.float32)

    f16 = np.float16
    f8 = ml_dtypes.float8_e4m3
    wall = np.ascontiguousarray(
        W.transpose(1, 0, 2).reshape(IN_DIM, CD)).astype(f16)
    bflat = np.ascontiguousarray(b.reshape(1, CD).reshape(NPAIR, 128).T).astype(np.float32)
    ident = np.eye(128, dtype=f16)
    blkd = np.zeros((128, NPAIR * 8), dtype=f16)
    seld = np.zeros((8, NPAIR * 128), dtype=f16)
    for t in range(NPAIR):
        for h in range(2):
            c = 2 * t + h
            blkd[h * 64:(h + 1) * 64, t * 8 + c] = 1.0
            seld[c, t * 128 + h * 64:t * 128 + (h + 1) * 64] = 1.0
    onesd = np.ones((1, 128), dtype=f16)
    wo16 = W_o.astype(f16)
    bias16 = bias.reshape(1, OUT).astype(f16)

    in_maps = []
    for r in range(NCORES):
        rows = slice(r * NL, (r + 1) * NL)
        in_maps.append({
            "featT": np.ascontiguousarray(features[rows].T).astype(f16),
            "wall": wall,
            "bflat": bflat,
            "maskT": np.ascontiguousarray(adj[rows].T).astype(f16),
            "wo": wo16,
            "biasd": bias16,
            "ident": ident,
            "blkd": blkd,
            "seld": seld,
            "onesd": onesd,
        })
    return in_maps


_NC_CACHE = []


def _get_nc():
    if not _NC_CACHE:
        _NC_CACHE.append(_build_nc())
    return _NC_CACHE[0]


def run(inputs, trace=False, **kwargs):
    nc = _get_nc()
    in_maps = _make_in_maps(**inputs)
    res = run_bass_kernel_spmd(nc, in_maps, core_ids=list(range(NCORES)),
                               trace=trace, **kwargs)
    out = np.concatenate([res.results[r]["outd"] for r in range(NCORES)],
                         axis=0).astype(np.float32)
    return out, res


def kernel(features, adj, W, b, W_o, bias):
    out, _ = run(dict(features=features, adj=adj, W=W, b=b, W_o=W_o, bias=bias))
    return out


# revision 21
# speedup vs baseline: 1.3468x; 1.0384x over previous
"""Disen-GCN (8-channel routing attention GNN) on 8 TRN2 NeuronCores.

Row-parallel sharding: core r owns node rows [r*512, (r+1)*512).
Per routing iteration:
  phase1: L[c][j, i] = z[c,j] . z[c,i]  fp8e4 DoubleRow matmul (K=64
          folded 2-per-partition onto 32 partitions; 256 PE cyc/instr)
  exp:    E8[:, c*512+i] = exp(L)       (ACT, PSUM->SBUF fp16)
  smax:   S = sum_c E8 (v-add on Pool); Q = mask * 1/S (DVE)
  R:      R = E8 * broadcast(Q)  (ch 0-5 DVE, ch 6-7 Pool)
  phase3: agg^T[c][d, i] += znat[c][j,:]^T @ R[c]  (fp16 PE, PSUM acc)
  norm:   z = l2norm(z + agg); re-quantize z to fp8 fold layout
  ship:   one merged AllGather (fp16 znat rows + bitcast fp8 zT fold)
          via internal shared DRAM.
Final: out = concat_c(z) @ W_o + bias.
"""

import numpy as np
from contextlib import ExitStack

from concourse import bacc, bass, tile, mybir
from concourse.bass_utils import run_bass_kernel_spmd
from concourse import dve_ops as _dvo
from concourse.dve_spec import Spec, Src0, Src1, C0, C1, AluOp, Bin
from concourse.dve_spec import lower as _dve_lower
from concourse.dve_ops import DveOp, DveOpSpec


def _ref_qrecip(in0, in1, c0, c1, c2):
    x = np.asarray(in0, dtype=np.float32)
    not_x = (~x.view(np.int32)).view(np.float32)
    y0 = not_x * np.float32(c0)
    y1 = y0 * (np.float32(c1) - x * y0)
    return y1 * np.asarray(in1, dtype=np.float32)


def _make_qrecip():
    # Q = mask * approx(1/S): BITWISE_NOT exponent-flip seed + one
    # Newton-Raphson pass (~0.4% rel err, plenty for fp16 weights),
    # fused with the mask multiply. 6 ALU stages.
    not_x = Bin(AluOp.BITWISE_NOT, Src0, Src0)
    y0 = not_x * C0
    y1 = y0 * (C1 - Src0 * y0)
    spec = Spec(body=y1 * Src1, reference=_ref_qrecip)
    name = "QRECIP_ANT"
    opcode = _dvo._CUSTOM_DVE_ROW_BASE + len(_dvo.OPS)
    assert opcode < 0x20
    shas = {}
    for ver in ("v3", "v4"):
        s = DveOpSpec(name=name, opcode=opcode, uops=_dve_lower(spec, ver=ver),
                      rd1_en=True)
        shas[ver] = s.sha(ver)
    op = DveOp(name, spec, subdim=False, uops_sha=shas,
               perf_en={"v3": True, "v4": True})
    _dvo.OPS.append(op)
    _dvo._SUB_OPCODE_FOR_NAME[name] = opcode
    _dvo.CUSTOM_DVE_SPECS[name] = spec
    return op


QRECIP = _make_qrecip()
QRECIP_C0 = float(_dvo.RECIP_APPROX_FAST_CONSTS["s0"])
QRECIP_C1 = float(_dvo.RECIP_APPROX_FAST_CONSTS["s1"])

F32 = mybir.dt.float32
F16 = mybir.dt.float16
F8 = mybir.dt.float8e4
DR = mybir.MatmulPerfMode.DoubleRow

N = 4096
C = 8
IN_DIM = 256
D = 64
OUT = 128
ITERS = 4
NCORES = 8
NL = N // NCORES          # 512 local rows
CD = C * D                # 512
NJT = N // 128            # 32 j-tiles
NPAIR = C // 2            # 4 channel-pair tiles
AF = mybir.ActivationFunctionType
RG = [list(range(NCORES))]
PIPE_DEPTH = 2            # phase3 lags the softmax by this many j-tiles
E8W = C * NL              # 4096: fused E/R tile width


def _build_nc():
    nc = bacc.Bacc(
        "TRN2", target_bir_lowering=False, debug=False, num_devices=NCORES
    )
    featT = nc.dram_tensor("featT", [IN_DIM, NL], F16, kind="ExternalInput").ap()
    wall = nc.dram_tensor("wall", [IN_DIM, CD], F16, kind="ExternalInput").ap()
    bflat = nc.dram_tensor("bflat", [128, NPAIR], F32, kind="ExternalInput").ap()
    maskT = nc.dram_tensor("maskT", [N, NL], F16, kind="ExternalInput").ap()
    wo = nc.dram_tensor("wo", [CD, OUT], F16, kind="ExternalInput").ap()
    biasd = nc.dram_tensor("biasd", [1, OUT], F16, kind="ExternalInput").ap()
    ident = nc.dram_tensor("ident", [128, 128], F16, kind="ExternalInput").ap()
    blkd = nc.dram_tensor("blkd", [128, NPAIR * 8], F16, kind="ExternalInput").ap()
    seld = nc.dram_tensor("seld", [8, NPAIR * 128], F16, kind="ExternalInput").ap()
    onesd = nc.dram_tensor("onesd", [1, 128], F16, kind="ExternalInput").ap()
    outd = nc.dram_tensor("outd", [NL, OUT], F32, kind="ExternalOutput").ap()

    with tile.TileContext(nc) as tc:
        _body(nc, tc, featT, wall, bflat, maskT, wo, biasd, ident, blkd, seld,
              onesd, outd)
    nc.compile()
    return nc


def _body(nc, tc, featT, wall, bflat, maskT, wo, biasd, ident, blkd, seld,
          onesd, outd):
    ctx = ExitStack()
    const = ctx.enter_context(tc.tile_pool(name="const", bufs=1))
    big = ctx.enter_context(tc.tile_pool(name="big", bufs=1))
    work = ctx.enter_context(tc.tile_pool(name="work", bufs=1))
    psum = ctx.enter_context(tc.tile_pool(name="psum", bufs=1, space="PSUM"))
    dram = ctx.enter_context(tc.tile_pool(name="dram", bufs=1, space="DRAM"))

    def loadc(dr_ap, shape, name):
        dst = const.tile(shape, F16, tag=name, bufs=1, name=name)
        nc.sync.dma_start(out=dst, in_=dr_ap)
        return dst

    # ---- constants / weights (fp16 already on host) ----
    ident16 = loadc(ident, [128, 128], "ident16")
    blkd16 = loadc(blkd, [128, NPAIR * 8], "blkd16")
    sel16 = loadc(seld, [8, NPAIR * 128], "sel16")
    ones16 = loadc(onesd, [1, 128], "ones16")
    bT32 = const.tile([128, NPAIR], F32, tag="bT32", bufs=1, name="bT32")
    nc.sync.dma_start(out=bT32, in_=bflat)
    bias16 = loadc(biasd, [1, OUT], "bias16")
    zeros16 = const.tile([1, NL], F16, tag="zeros16", bufs=1, name="zeros16")
    nc.vector.memset(zeros16, 0.0)

    featT16 = const.tile([128, 2 * NL], F16, tag="featT16", bufs=1, name="featT16")
    nc.sync.dma_start(
        out=featT16.rearrange("p (k i) -> p k i", k=2),
        in_=featT.rearrange("(k p) i -> p k i", p=128))
    w016 = const.tile([128, 2 * CD], F16, tag="w016", bufs=1, name="w016")
    nc.sync.dma_start(
        out=w016.rearrange("p (k i) -> p k i", k=2),
        in_=wall.rearrange("(k p) i -> p k i", p=128))
    wo16 = const.tile([128, 4 * OUT], F16, tag="wo16", bufs=1, name="wo16")
    nc.sync.dma_start(
        out=wo16.rearrange("p (k i) -> p k i", k=4),
        in_=wo.rearrange("(k p) i -> p k i", p=128))

    # ---- resident mask (fp16): mask16[:, jt*512 + i] = adj[i_global, j] ----
    # per-jt DMAs on the gpsimd queue so they don't block the ship DMAs
    mask16 = big.tile([128, NJT * NL], F16, tag="mask16", bufs=1, name="mask16")
    for jt in range(NJT):
        nc.gpsimd.dma_start(
            out=mask16[:, jt * NL:(jt + 1) * NL],
            in_=maskT[jt * 128:(jt + 1) * 128, :])

    # ---- skew-absorbing barrier: a tiny AllGather so the first real
    # collective doesn't pay the cross-core launch skew ----
    sk_in = dram.tile([1, 64], F16, tag="skin", bufs=1, name="sk_in")
    sk_src = const.tile([1, 64], F16, tag="sksrc", bufs=1, name="sk_src")
    nc.vector.memset(sk_src, 0.0)
    nc.sync.dma_start(out=sk_in, in_=sk_src)
    sk_out = dram.tile([NCORES, 64], F16, tag="skout", bufs=1,
                       addr_space="Shared", name="sk_out")
    nc.gpsimd.collective_compute(
        "AllGather", mybir.AluOpType.bypass, replica_groups=RG,
        ins=[sk_in.opt()], outs=[sk_out.opt()])

    # ---- resident full z: fp8 folded zT (phase1) + fp16 natural (phase3) ----
    # zT8f: channel c at partitions [(c%2)*64, +32), cols
    #   (c//2)*8*1024 + r*1024 + s*512 + i  (r=rank, s=fold slot: d=s*32+p)
    zT8f = big.tile([128, 4 * 8 * 1024], F8, tag="zT8f", bufs=1, name="zT8f")
    znat16 = big.tile([128, NJT * CD], F16, tag="znat16", bufs=1, name="znat16")

    def normalize_and_rows(zpre, it, want_nat=True):
        """zpre: 4 SBUF fp16 tiles [128, NL] (z_T rows layout, pre-norm).
        Returns (zrows, natrows): l2-normalized rows in both layouts."""
        nrm = psum.tile([8, NL], F32, tag="L", bufs=2, name=f"nrm_{it}")
        for t in range(NPAIR):
            sq = work.tile([128, NL], F16, tag="sq", bufs=2, name=f"sq_{it}_{t}")
            nc.vector.tensor_mul(out=sq, in0=zpre[t], in1=zpre[t])
            nc.tensor.matmul(out=nrm, lhsT=blkd16[:, t * 8:(t + 1) * 8], rhs=sq,
                             start=(t == 0), stop=(t == NPAIR - 1))
        rsq = work.tile([8, NL], F16, tag="rsq", bufs=2, name=f"rsq_{it}")
        # rsqrt straight from PSUM (sumsq of this data is bounded >> 1e-12,
        # so the reference's clamp is a numeric no-op)
        nc.scalar.activation(out=rsq, in_=nrm, func=AF.Abs_reciprocal_sqrt)
        zrows = []
        for t in range(NPAIR):
            bc = psum.tile([128, NL], F32, tag="L", bufs=2, name=f"bc_{it}_{t}")
            nc.tensor.matmul(out=bc, lhsT=sel16[:, t * 128:(t + 1) * 128],
                             rhs=rsq, start=True, stop=True)
            zr = work.tile([128, NL], F16, tag="zrows", bufs=8,
                           name=f"zrows_{it}_{t}")
            nc.vector.tensor_mul(out=zr, in0=zpre[t], in1=bc)
            zrows.append(zr)
        if not want_nat:
            return zrows, None
        natrows = [work.tile([128, CD], F16, tag="natrows", bufs=4,
                             name=f"natr_{it}_{ib}") for ib in range(4)]
        for t in range(NPAIR):
            for ib in range(4):
                tp = psum.tile([128, 128], F16, tag="L", bufs=2,
                               name=f"tp_{it}_{t}_{ib}")
                nc.tensor.transpose(out=tp,
                                    in_=zrows[t][:, ib * 128:(ib + 1) * 128],
                                    identity=ident16)
                nc.vector.tensor_copy(
                    out=natrows[ib][:, t * 128:(t + 1) * 128], in_=tp)
        return zrows, natrows

    def ship_all(zrows, natrows, it):
        """Two AllGathers: CC-A ships the fp8 folded zT (256KB,
        phase1-critical) first, CC-B the fp16 nat rows (512KB, phase3
        lags behind PIPE_DEPTH so it can land later). Returns the local
        folded rhs tile zfold8: channel c at partitions [(c%2)*64, +32),
        cols (c//2)*1024 + s*512 + i, where z[c, i, d] sits at fold
        partition p=d%32, slot s=d//32."""
        agA_in = dram.tile([64, 4096], F8, tag="aginA", bufs=2,
                           name=f"aginA_{it}")
        for t in range(NPAIR):
            z8 = work.tile([128, NL], F8, tag="z8", bufs=4,
                           name=f"z8_{it}_{t}")
            # fp16 -> fp8 copy on ACT (Copy is in the exp table set)
            nc.scalar.activation(out=z8, in_=zrows[t], func=AF.Copy)
            for h in range(2):
                c = 2 * t + h
                b = c % 2
                cc = c // 2
                # fold row fr=b*32+p, byte col cc*1024+s*512+i
                nc.sync.dma_start(
                    out=agA_in[b * 32:(b + 1) * 32,
                               cc * 1024:(cc + 1) * 1024]
                        .rearrange("p (s i) -> s p i", s=2),
                    in_=z8[h * 64:(h + 1) * 64, :])
        agB_in = dram.tile([NL, CD], F16, tag="aginB", bufs=2,
                           name=f"aginB_{it}")
        for ib in range(4):
            nc.sync.dma_start(out=agB_in[ib * 128:(ib + 1) * 128, :],
                              in_=natrows[ib])
        # local folded rhs for phase1 (round-trip through agA_in)
        zfold8 = work.tile([128, 4 * 1024], F8, tag="zfold8", bufs=2,
                           name=f"zfold8_{it}")
        nc.sync.dma_start(out=zfold8[0:32, :], in_=agA_in[0:32, :])
        nc.sync.dma_start(out=zfold8[64:96, :], in_=agA_in[32:64, :])
        agA_out = dram.tile([NCORES * 64, 4096], F8, tag="agoutA", bufs=2,
                            addr_space="Shared", name=f"agoutA_{it}")
        nc.gpsimd.collective_compute(
            "AllGather", mybir.AluOpType.bypass, replica_groups=RG,
            ins=[agA_in.opt()], outs=[agA_out.opt()])
        agB_out = dram.tile([NCORES * NL, CD], F16, tag="agoutB", bufs=2,
                            addr_space="Shared", name=f"agoutB_{it}")
        nc.gpsimd.collective_compute(
            "AllGather", mybir.AluOpType.bypass, replica_groups=RG,
            ins=[agB_in.opt()], outs=[agB_out.opt()])
        agA_v = agA_out.rearrange("(r b p) f -> r b p f", r=NCORES, b=2)
        for r in range(NCORES):
            # fold readback first (phase1-critical)
            for b in range(2):
                nc.sync.dma_start(
                    out=zT8f[b * 64:b * 64 + 32, :]
                        .rearrange("p (cc rr f) -> p cc rr f", cc=4, rr=8)
                        [:, :, r, :],
                    in_=agA_v[r, b].rearrange("p (cc f) -> p cc f", cc=4))
        agB_v = agB_out.rearrange("(r q) d -> r q d", r=NCORES)
        for r in range(NCORES):
            nc.gpsimd.dma_start(
                out=znat16[:, r * 4 * CD:(r + 1) * 4 * CD]
                    .rearrange("p (pb d) -> p pb d", pb=4),
                in_=agB_v[r].rearrange("(pb p) d -> p pb d", pb=4))
        return zfold8

    # ===== phase 0: z0 = l2norm(features @ W + b), built in z_T layout =====
    zpre0 = []
    for t in range(NPAIR):
        zp = psum.tile([128, NL], F32, tag="L", bufs=2, name=f"zp_{t}")
        for kt in range(2):
            nc.tensor.matmul(
                out=zp,
                lhsT=w016[:, kt * CD + t * 128:kt * CD + (t + 1) * 128],
                rhs=featT16[:, kt * NL:(kt + 1) * NL],
                start=(kt == 0), stop=(kt == 1))
        zt = work.tile([128, NL], F16, tag="zpre0", bufs=5, name=f"zpre0_{t}")
        nc.scalar.activation(out=zt, in_=zp, func=AF.Identity,
                             bias=bT32[:, t:t + 1])
        zpre0.append(zt)
    zrows, natrows = normalize_and_rows(zpre0, it=-1)
    zfold8 = ship_all(zrows, natrows, it=-1)

    # ================= routing iterations =================
    for it in range(ITERS):
        agg = [psum.tile([128, NL], F32, tag="agg", bufs=4, name=f"agg_{it}_{t}")
               for t in range(NPAIR)]
        for t in range(NPAIR):
            # zero-fill the whole bank once so both col-tiled halves can
            # accumulate with start=False (start clears the full bank)
            nc.tensor.matmul(out=agg[t], lhsT=ones16, rhs=zeros16,
                             start=True, stop=False)
        pending = []

        def ph3_emit(pjt, pR8, c0, c1):
            for c in range(c0, c1):
                t, h = c // 2, c % 2
                nc.tensor.matmul(
                    out=agg[t][h * 64:(h + 1) * 64, :],
                    lhsT=znat16[:, pjt * CD + c * 64:pjt * CD + (c + 1) * 64],
                    rhs=pR8[:, c * NL:(c + 1) * NL],
                    start=False, stop=False,
                    tile_position=(0, h * 64))

        for jt in range(NJT):
            drain = pending.pop(0) if len(pending) > PIPE_DEPTH else None
            E8 = work.tile([128, E8W], F16, tag="E", bufs=3,
                           name=f"E8_{it}_{jt}")
            for t in range(NPAIR):
                L2 = psum.tile([128, 2 * NL], F32, tag="L", bufs=2,
                               name=f"L2_{it}_{jt}_{t}")
                for h in range(2):
                    c = 2 * t + h
                    cb = (c % 2) * 64       # partition base
                    cc = c // 2             # column block
                    nc.tensor.matmul(
                        out=L2[:, h * NL:(h + 1) * NL],
                        lhsT=zT8f[cb:cb + 32,
                                  cc * 8192 + 0:cc * 8192 + 8192]
                            .rearrange("p (rr s i) -> p rr s i", rr=8, s=2)
                            [:, jt // 4, :, (jt % 4) * 128:(jt % 4 + 1) * 128],
                        rhs=zfold8[cb:cb + 32, cc * 1024:(cc + 1) * 1024]
                            .rearrange("p (s i) -> p s i", s=2),
                        start=True, stop=True, perf_mode=DR,
                        tile_position=(cb, 0))
                nc.scalar.activation(
                    out=E8[:, t * 2 * NL:(t + 1) * 2 * NL], in_=L2,
                    func=AF.Exp)
                # interleave aggregation matmuls of the lagged j-tile
                # between phase1 pairs so the in-order PE queue can fill
                # L2-slot waits with ready work
                if drain is not None:
                    ph3_emit(drain[0], drain[1], t * 2, (t + 1) * 2)
            # channel-softmax denominator: tree sum, all on DVE (running
            # it on Pool concurrently contends for the same SBUF tiles and
            # slows both engines down)
            u = work.tile([128, 2 * NL], F16, tag="s2", bufs=6,
                          name=f"u_{it}_{jt}")
            nc.vector.tensor_add(out=u, in0=E8[:, 0:1024], in1=E8[:, 1024:2048])
            v = work.tile([128, 2 * NL], F16, tag="s2", bufs=6,
                          name=f"v_{it}_{jt}")
            nc.vector.tensor_add(out=v, in0=E8[:, 2048:3072],
                                 in1=E8[:, 3072:4096])
            w = work.tile([128, 2 * NL], F16, tag="s2", bufs=6,
                          name=f"w_{it}_{jt}")
            nc.vector.tensor_add(out=w, in0=u, in1=v)
            S16 = work.tile([128, NL], F16, tag="S16", bufs=4,
                            name=f"S16_{it}_{jt}")
            nc.vector.tensor_add(out=S16, in0=w[:, 0:NL], in1=w[:, NL:])
            # Q = mask * 1/S in one fused custom-DVE op
            Q = work.tile([128, NL], F16, tag="Q", bufs=4, name=f"Q_{it}_{jt}")
            nc.vector._custom_dve(
                QRECIP, out=Q, in0=S16,
                in1=mask16[:, jt * NL:(jt + 1) * NL],
                s0=QRECIP_C0, s1=QRECIP_C1)
            # R8 = E8 * broadcast(Q) over the 8 channel blocks, one DVE op
            R8 = work.tile([128, E8W], F16, tag="R", bufs=PIPE_DEPTH + 1,
                           name=f"R8_{it}_{jt}")
            nc.vector.tensor_mul(
                out=R8.rearrange("p (c i) -> p c i", c=C),
                in0=E8.rearrange("p (c i) -> p c i", c=C),
                in1=Q.unsqueeze(1).broadcast_to([128, C, NL]))
            pending.append((jt, R8))
        for pjt, pR8 in pending:
            ph3_emit(pjt, pR8, 0, C)
        for t in range(NPAIR):
            # N=1 dummy stop: closes the sim accumulation group, no-op on HW
            nc.tensor.matmul(out=agg[t][:, 0:1], lhsT=ones16,
                             rhs=zeros16[:, 0:1], start=False, stop=True)
        # residual + renorm
        zpre = []
        for t in range(NPAIR):
            zq = work.tile([128, NL], F16, tag="zpre0", bufs=5,
                           name=f"zpre_{it}_{t}")
            nc.vector.tensor_add(out=zq, in0=zrows[t], in1=agg[t])
            zpre.append(zq)
        zrows, natrows = normalize_and_rows(zpre, it=it,
                                            want_nat=(it < ITERS - 1))
        if it < ITERS - 1:
            zfold8 = ship_all(zrows, natrows, it=it)

    # ================= output: h @ W_o + bias =================
    for ib in range(4):
        op = psum.tile([128, OUT], F32, tag="L", bufs=2, name=f"op_{ib}")
        for kt in range(4):
            nc.tensor.matmul(out=op,
                             lhsT=zrows[kt][:, ib * 128:(ib + 1) * 128],
                             rhs=wo16[:, kt * OUT:(kt + 1) * OUT],
                             start=(kt == 0), stop=False)
        nc.tensor.matmul(out=op, lhsT=ones16, rhs=bias16, start=False, stop=True)
        ot = work.tile([128, OUT], F32, tag="ot", bufs=2, name=f"ot_{ib}")
        nc.vector.tensor_copy(out=ot, in_=op)
        nc.sync.dma_start(out=outd[ib * 128:(ib + 1) * 128, :], in_=ot)

    ctx.close()


def _make_in_maps(features, adj, W, b, W_o, bias):
    import ml_dtypes
    features = np.asarray(features, dtype=np.float32)
    adj = np.asarray(adj, dtype=np.float32)
    W = np.asarray(W, dtype=np.float32)
    b = np.asarray(b, dtype=np.float32)
    W_o = np.asarray(W_o, dtype=np.float32)
    bias = np.asarray(bias, dtype=np.float32)

    f16 = np.float16
    f8 = ml_dtypes.float8_e4m3
    wall = np.ascontiguousarray(
        W.transpose(1, 0, 2).reshape(IN_DIM, CD)).astype(f16)
    bflat = np.ascontiguousarray(b.reshape(1, CD).reshape(NPAIR, 128).T).astype(np.float32)
    ident = np.eye(128, dtype=f16)
    blkd = np.zeros((128, NPAIR * 8), dtype=f16)
    seld = np.zeros((8, NPAIR * 128), dtype=f16)
    for t in range(NPAIR):
        for h in range(2):
            c = 2 * t + h
            blkd[h * 64:(h + 1) * 64, t * 8 + c] = 1.0
            seld[c, t * 128 + h * 64:t * 128 + (h + 1) * 64] = 1.0
    onesd = np.ones((1, 128), dtype=f16)
    wo16 = W_o.astype(f16)
    bias16 = bias.reshape(1, OUT).astype(f16)

    in_maps = []
    for r in range(NCORES):
        rows = slice(r * NL, (r + 1) * NL)
        in_maps.append({
            "featT": np.ascontiguousarray(features[rows].T).astype(f16),
            "wall": wall,
            "bflat": bflat,
            "maskT": np.ascontiguousarray(adj[rows].T).astype(f16),
            "wo": wo16,
            "biasd": bias16,
            "ident": ident,
            "blkd": blkd,
            "seld": seld,
            "onesd": onesd,
        })
    return in_maps


_NC_CACHE = []


def _get_nc():
    if not _NC_CACHE:
        _NC_CACHE.append(_build_nc())
    return _NC_CACHE[0]


def run(inputs, trace=False, **kwargs):
    nc = _get_nc()
    in_maps = _make_in_maps(**inputs)
    res = run_bass_kernel_spmd(nc, in_maps, core_ids=list(range(NCORES)),
                               trace=trace, **kwargs)
    out = np.concatenate([res.results[r]["outd"] for r in range(NCORES)],
                         axis=0).astype(np.float32)
    return out, res


def kernel(features, adj, W, b, W_o, bias):
    out, _ = run(dict(features=features, adj=adj, W=W, b=b, W_o=W_o, bias=bias))
    return out


# revision 22
# speedup vs baseline: 1.3483x; 1.0011x over previous
"""Disen-GCN (8-channel routing attention GNN) on 8 TRN2 NeuronCores.

Row-parallel sharding: core r owns node rows [r*512, (r+1)*512).
Per routing iteration:
  phase1: L[c][j, i] = z[c,j] . z[c,i]  fp8e4 DoubleRow matmul (K=64
          folded 2-per-partition onto 32 partitions; 256 PE cyc/instr)
  exp:    E8[:, c*512+i] = exp(L)       (ACT, PSUM->SBUF fp16)
  smax:   S = sum_c E8 (v-add on Pool); Q = mask * 1/S (DVE)
  R:      R = E8 * broadcast(Q)  (ch 0-5 DVE, ch 6-7 Pool)
  phase3: agg^T[c][d, i] += znat[c][j,:]^T @ R[c]  (fp16 PE, PSUM acc)
  norm:   z = l2norm(z + agg); re-quantize z to fp8 fold layout
  ship:   one merged AllGather (fp16 znat rows + bitcast fp8 zT fold)
          via internal shared DRAM.
Final: out = concat_c(z) @ W_o + bias.
"""

import numpy as np
from contextlib import ExitStack

from concourse import bacc, bass, tile, mybir
from concourse.bass_utils import run_bass_kernel_spmd
from concourse import dve_ops as _dvo
from concourse.dve_spec import Spec, Src0, Src1, C0, C1, AluOp, Bin
from concourse.dve_spec import lower as _dve_lower
from concourse.dve_ops import DveOp, DveOpSpec


def _ref_qrecip(in0, in1, c0, c1, c2):
    x = np.asarray(in0, dtype=np.float32)
    not_x = (~x.view(np.int32)).view(np.float32)
    y0 = not_x * np.float32(c0)
    y1 = y0 * (np.float32(c1) - x * y0)
    return y1 * np.asarray(in1, dtype=np.float32)


def _make_qrecip():
    # Q = mask * approx(1/S): BITWISE_NOT exponent-flip seed + one
    # Newton-Raphson pass (~0.4% rel err, plenty for fp16 weights),
    # fused with the mask multiply. 6 ALU stages.
    not_x = Bin(AluOp.BITWISE_NOT, Src0, Src0)
    y0 = not_x * C0
    y1 = y0 * (C1 - Src0 * y0)
    spec = Spec(body=y1 * Src1, reference=_ref_qrecip)
    name = "QRECIP_ANT"
    opcode = _dvo._CUSTOM_DVE_ROW_BASE + len(_dvo.OPS)
    assert opcode < 0x20
    shas = {}
    for ver in ("v3", "v4"):
        s = DveOpSpec(name=name, opcode=opcode, uops=_dve_lower(spec, ver=ver),
                      rd1_en=True)
        shas[ver] = s.sha(ver)
    op = DveOp(name, spec, subdim=False, uops_sha=shas,
               perf_en={"v3": True, "v4": True})
    _dvo.OPS.append(op)
    _dvo._SUB_OPCODE_FOR_NAME[name] = opcode
    _dvo.CUSTOM_DVE_SPECS[name] = spec
    return op


QRECIP = _make_qrecip()
QRECIP_C0 = float(_dvo.RECIP_APPROX_FAST_CONSTS["s0"])
QRECIP_C1 = float(_dvo.RECIP_APPROX_FAST_CONSTS["s1"])

F32 = mybir.dt.float32
F16 = mybir.dt.float16
F8 = mybir.dt.float8e4
DR = mybir.MatmulPerfMode.DoubleRow

N = 4096
C = 8
IN_DIM = 256
D = 64
OUT = 128
ITERS = 4
NCORES = 8
NL = N // NCORES          # 512 local rows
CD = C * D                # 512
NJT = N // 128            # 32 j-tiles
NPAIR = C // 2            # 4 channel-pair tiles
AF = mybir.ActivationFunctionType
RG = [list(range(NCORES))]
PIPE_DEPTH = 2            # phase3 lags the softmax by this many j-tiles
E8W = C * NL              # 4096: fused E/R tile width


def _build_nc():
    nc = bacc.Bacc(
        "TRN2", target_bir_lowering=False, debug=False, num_devices=NCORES
    )
    featT = nc.dram_tensor("featT", [IN_DIM, NL], F16, kind="ExternalInput").ap()
    wall = nc.dram_tensor("wall", [IN_DIM, CD], F16, kind="ExternalInput").ap()
    bflat = nc.dram_tensor("bflat", [128, NPAIR], F32, kind="ExternalInput").ap()
    maskT = nc.dram_tensor("maskT", [N, NL], F16, kind="ExternalInput").ap()
    wo = nc.dram_tensor("wo", [CD, OUT], F16, kind="ExternalInput").ap()
    biasd = nc.dram_tensor("biasd", [1, OUT], F16, kind="ExternalInput").ap()
    ident = nc.dram_tensor("ident", [128, 128], F16, kind="ExternalInput").ap()
    blkd = nc.dram_tensor("blkd", [128, NPAIR * 8], F16, kind="ExternalInput").ap()
    seld = nc.dram_tensor("seld", [8, NPAIR * 128], F16, kind="ExternalInput").ap()
    onesd = nc.dram_tensor("onesd", [1, 128], F16, kind="ExternalInput").ap()
    outd = nc.dram_tensor("outd", [NL, OUT], F32, kind="ExternalOutput").ap()

    with tile.TileContext(nc) as tc:
        _body(nc, tc, featT, wall, bflat, maskT, wo, biasd, ident, blkd, seld,
              onesd, outd)
    nc.compile()
    return nc


def _body(nc, tc, featT, wall, bflat, maskT, wo, biasd, ident, blkd, seld,
          onesd, outd):
    ctx = ExitStack()
    const = ctx.enter_context(tc.tile_pool(name="const", bufs=1))
    big = ctx.enter_context(tc.tile_pool(name="big", bufs=1))
    work = ctx.enter_context(tc.tile_pool(name="work", bufs=1))
    psum = ctx.enter_context(tc.tile_pool(name="psum", bufs=1, space="PSUM"))
    dram = ctx.enter_context(tc.tile_pool(name="dram", bufs=1, space="DRAM"))

    def loadc(dr_ap, shape, name):
        dst = const.tile(shape, F16, tag=name, bufs=1, name=name)
        nc.sync.dma_start(out=dst, in_=dr_ap)
        return dst

    # ---- constants / weights (fp16 already on host) ----
    ident16 = loadc(ident, [128, 128], "ident16")
    blkd16 = loadc(blkd, [128, NPAIR * 8], "blkd16")
    sel16 = loadc(seld, [8, NPAIR * 128], "sel16")
    ones16 = loadc(onesd, [1, 128], "ones16")
    bT32 = const.tile([128, NPAIR], F32, tag="bT32", bufs=1, name="bT32")
    nc.sync.dma_start(out=bT32, in_=bflat)
    bias16 = loadc(biasd, [1, OUT], "bias16")
    zeros16 = const.tile([1, NL], F16, tag="zeros16", bufs=1, name="zeros16")
    nc.vector.memset(zeros16, 0.0)

    featT16 = const.tile([128, 2 * NL], F16, tag="featT16", bufs=1, name="featT16")
    nc.sync.dma_start(
        out=featT16.rearrange("p (k i) -> p k i", k=2),
        in_=featT.rearrange("(k p) i -> p k i", p=128))
    w016 = const.tile([128, 2 * CD], F16, tag="w016", bufs=1, name="w016")
    nc.sync.dma_start(
        out=w016.rearrange("p (k i) -> p k i", k=2),
        in_=wall.rearrange("(k p) i -> p k i", p=128))
    wo16 = const.tile([128, 4 * OUT], F16, tag="wo16", bufs=1, name="wo16")
    nc.sync.dma_start(
        out=wo16.rearrange("p (k i) -> p k i", k=4),
        in_=wo.rearrange("(k p) i -> p k i", p=128))

    # ---- resident mask (fp16): mask16[:, jt*512 + i] = adj[i_global, j] ----
    mask16 = big.tile([128, NJT * NL], F16, tag="mask16", bufs=1, name="mask16")

    # ---- resident full z: fp8 folded zT (phase1) + fp16 natural (phase3) ----
    # zT8f: channel c at partitions [(c%2)*64, +32), cols
    #   (c//2)*8*1024 + r*1024 + s*512 + i  (r=rank, s=fold slot: d=s*32+p)
    zT8f = big.tile([128, 4 * 8 * 1024], F8, tag="zT8f", bufs=1, name="zT8f")
    znat16 = big.tile([128, NJT * CD], F16, tag="znat16", bufs=1, name="znat16")

    def normalize_and_rows(zpre, it, want_nat=True):
        """zpre: 4 SBUF fp16 tiles [128, NL] (z_T rows layout, pre-norm).
        Returns (zrows, natrows): l2-normalized rows in both layouts."""
        nrm = psum.tile([8, NL], F32, tag="L", bufs=2, name=f"nrm_{it}")
        for t in range(NPAIR):
            sq = work.tile([128, NL], F16, tag="sq", bufs=2, name=f"sq_{it}_{t}")
            nc.vector.tensor_mul(out=sq, in0=zpre[t], in1=zpre[t])
            nc.tensor.matmul(out=nrm, lhsT=blkd16[:, t * 8:(t + 1) * 8], rhs=sq,
                             start=(t == 0), stop=(t == NPAIR - 1))
        rsq = work.tile([8, NL], F16, tag="rsq", bufs=2, name=f"rsq_{it}")
        # rsqrt straight from PSUM (sumsq of this data is bounded >> 1e-12,
        # so the reference's clamp is a numeric no-op)
        nc.scalar.activation(out=rsq, in_=nrm, func=AF.Abs_reciprocal_sqrt)
        zrows = []
        for t in range(NPAIR):
            bc = psum.tile([128, NL], F32, tag="L", bufs=2, name=f"bc_{it}_{t}")
            nc.tensor.matmul(out=bc, lhsT=sel16[:, t * 128:(t + 1) * 128],
                             rhs=rsq, start=True, stop=True)
            zr = work.tile([128, NL], F16, tag="zrows", bufs=8,
                           name=f"zrows_{it}_{t}")
            nc.vector.tensor_mul(out=zr, in0=zpre[t], in1=bc)
            zrows.append(zr)
        if not want_nat:
            return zrows, None
        natrows = [work.tile([128, CD], F16, tag="natrows", bufs=4,
                             name=f"natr_{it}_{ib}") for ib in range(4)]
        for t in range(NPAIR):
            for ib in range(4):
                tp = psum.tile([128, 128], F16, tag="L", bufs=2,
                               name=f"tp_{it}_{t}_{ib}")
                nc.tensor.transpose(out=tp,
                                    in_=zrows[t][:, ib * 128:(ib + 1) * 128],
                                    identity=ident16)
                nc.vector.tensor_copy(
                    out=natrows[ib][:, t * 128:(t + 1) * 128], in_=tp)
        return zrows, natrows

    def ship_all(zrows, natrows, it):
        """Two AllGathers: CC-A ships the fp8 folded zT (256KB,
        phase1-critical) first, CC-B the fp16 nat rows (512KB, phase3
        lags behind PIPE_DEPTH so it can land later). Returns the local
        folded rhs tile zfold8: channel c at partitions [(c%2)*64, +32),
        cols (c//2)*1024 + s*512 + i, where z[c, i, d] sits at fold
        partition p=d%32, slot s=d//32."""
        agA_in = dram.tile([64, 4096], F8, tag="aginA", bufs=2,
                           name=f"aginA_{it}")
        for t in range(NPAIR):
            z8 = work.tile([128, NL], F8, tag="z8", bufs=4,
                           name=f"z8_{it}_{t}")
            # fp16 -> fp8 copy on ACT (Copy is in the exp table set)
            nc.scalar.activation(out=z8, in_=zrows[t], func=AF.Copy)
            for h in range(2):
                c = 2 * t + h
                b = c % 2
                cc = c // 2
                # fold row fr=b*32+p, byte col cc*1024+s*512+i
                nc.sync.dma_start(
                    out=agA_in[b * 32:(b + 1) * 32,
                               cc * 1024:(cc + 1) * 1024]
                        .rearrange("p (s i) -> s p i", s=2),
                    in_=z8[h * 64:(h + 1) * 64, :])
        agB_in = dram.tile([NL, CD], F16, tag="aginB", bufs=2,
                           name=f"aginB_{it}")
        for ib in range(4):
            nc.sync.dma_start(out=agB_in[ib * 128:(ib + 1) * 128, :],
                              in_=natrows[ib])
        # local folded rhs for phase1 (round-trip through agA_in)
        zfold8 = work.tile([128, 4 * 1024], F8, tag="zfold8", bufs=2,
                           name=f"zfold8_{it}")
        nc.sync.dma_start(out=zfold8[0:32, :], in_=agA_in[0:32, :])
        nc.sync.dma_start(out=zfold8[64:96, :], in_=agA_in[32:64, :])
        agA_out = dram.tile([NCORES * 64, 4096], F8, tag="agoutA", bufs=2,
                            addr_space="Shared", name=f"agoutA_{it}")
        nc.gpsimd.collective_compute(
            "AllGather", mybir.AluOpType.bypass, replica_groups=RG,
            ins=[agA_in.opt()], outs=[agA_out.opt()])
        agB_out = dram.tile([NCORES * NL, CD], F16, tag="agoutB", bufs=2,
                            addr_space="Shared", name=f"agoutB_{it}")
        nc.gpsimd.collective_compute(
            "AllGather", mybir.AluOpType.bypass, replica_groups=RG,
            ins=[agB_in.opt()], outs=[agB_out.opt()])
        agA_v = agA_out.rearrange("(r b p) f -> r b p f", r=NCORES, b=2)
        for r in range(NCORES):
            # fold readback first (phase1-critical)
            for b in range(2):
                nc.sync.dma_start(
                    out=zT8f[b * 64:b * 64 + 32, :]
                        .rearrange("p (cc rr f) -> p cc rr f", cc=4, rr=8)
                        [:, :, r, :],
                    in_=agA_v[r, b].rearrange("p (cc f) -> p cc f", cc=4))
        agB_v = agB_out.rearrange("(r q) d -> r q d", r=NCORES)
        for r in range(NCORES):
            nc.gpsimd.dma_start(
                out=znat16[:, r * 4 * CD:(r + 1) * 4 * CD]
                    .rearrange("p (pb d) -> p pb d", pb=4),
                in_=agB_v[r].rearrange("(pb p) d -> p pb d", pb=4))
        return zfold8

    # ===== phase 0: z0 = l2norm(features @ W + b), built in z_T layout =====
    zpre0 = []
    for t in range(NPAIR):
        zp = psum.tile([128, NL], F32, tag="L", bufs=2, name=f"zp_{t}")
        for kt in range(2):
            nc.tensor.matmul(
                out=zp,
                lhsT=w016[:, kt * CD + t * 128:kt * CD + (t + 1) * 128],
                rhs=featT16[:, kt * NL:(kt + 1) * NL],
                start=(kt == 0), stop=(kt == 1))
        zt = work.tile([128, NL], F16, tag="zpre0", bufs=5, name=f"zpre0_{t}")
        nc.scalar.activation(out=zt, in_=zp, func=AF.Identity,
                             bias=bT32[:, t:t + 1])
        zpre0.append(zt)
    zrows, natrows = normalize_and_rows(zpre0, it=-1)
    zfold8 = ship_all(zrows, natrows, it=-1)
    # mask loads after the first ship's collectives so their gpsimd-queue
    # triggers don't delay the startup-critical CC launch; the DMAs land
    # well before the first QRECIP needs them
    for jt in range(NJT):
        nc.gpsimd.dma_start(
            out=mask16[:, jt * NL:(jt + 1) * NL],
            in_=maskT[jt * 128:(jt + 1) * 128, :])

    # ================= routing iterations =================
    for it in range(ITERS):
        agg = [psum.tile([128, NL], F32, tag="agg", bufs=4, name=f"agg_{it}_{t}")
               for t in range(NPAIR)]
        for t in range(NPAIR):
            # zero-fill the whole bank once so both col-tiled halves can
            # accumulate with start=False (start clears the full bank)
            nc.tensor.matmul(out=agg[t], lhsT=ones16, rhs=zeros16,
                             start=True, stop=False)
        pending = []

        def ph3_emit(pjt, pR8, c0, c1):
            for c in range(c0, c1):
                t, h = c // 2, c % 2
                nc.tensor.matmul(
                    out=agg[t][h * 64:(h + 1) * 64, :],
                    lhsT=znat16[:, pjt * CD + c * 64:pjt * CD + (c + 1) * 64],
                    rhs=pR8[:, c * NL:(c + 1) * NL],
                    start=False, stop=False,
                    tile_position=(0, h * 64))

        for jt in range(NJT):
            drain = pending.pop(0) if len(pending) > PIPE_DEPTH else None
            E8 = work.tile([128, E8W], F16, tag="E", bufs=3,
                           name=f"E8_{it}_{jt}")
            for t in range(NPAIR):
                L2 = psum.tile([128, 2 * NL], F32, tag="L", bufs=2,
                               name=f"L2_{it}_{jt}_{t}")
                for h in range(2):
                    c = 2 * t + h
                    cb = (c % 2) * 64       # partition base
                    cc = c // 2             # column block
                    nc.tensor.matmul(
                        out=L2[:, h * NL:(h + 1) * NL],
                        lhsT=zT8f[cb:cb + 32,
                                  cc * 8192 + 0:cc * 8192 + 8192]
                            .rearrange("p (rr s i) -> p rr s i", rr=8, s=2)
                            [:, jt // 4, :, (jt % 4) * 128:(jt % 4 + 1) * 128],
                        rhs=zfold8[cb:cb + 32, cc * 1024:(cc + 1) * 1024]
                            .rearrange("p (s i) -> p s i", s=2),
                        start=True, stop=True, perf_mode=DR,
                        tile_position=(cb, 0))
                nc.scalar.activation(
                    out=E8[:, t * 2 * NL:(t + 1) * 2 * NL], in_=L2,
                    func=AF.Exp)
                # interleave aggregation matmuls of the lagged j-tile
                # between phase1 pairs so the in-order PE queue can fill
                # L2-slot waits with ready work
                if drain is not None:
                    ph3_emit(drain[0], drain[1], t * 2, (t + 1) * 2)
            # channel-softmax denominator: tree sum, all on DVE (running
            # it on Pool concurrently contends for the same SBUF tiles and
            # slows both engines down)
            u = work.tile([128, 2 * NL], F16, tag="s2", bufs=6,
                          name=f"u_{it}_{jt}")
            nc.vector.tensor_add(out=u, in0=E8[:, 0:1024], in1=E8[:, 1024:2048])
            v = work.tile([128, 2 * NL], F16, tag="s2", bufs=6,
                          name=f"v_{it}_{jt}")
            nc.vector.tensor_add(out=v, in0=E8[:, 2048:3072],
                                 in1=E8[:, 3072:4096])
            w = work.tile([128, 2 * NL], F16, tag="s2", bufs=6,
                          name=f"w_{it}_{jt}")
            nc.vector.tensor_add(out=w, in0=u, in1=v)
            S16 = work.tile([128, NL], F16, tag="S16", bufs=4,
                            name=f"S16_{it}_{jt}")
            nc.vector.tensor_add(out=S16, in0=w[:, 0:NL], in1=w[:, NL:])
            # Q = mask * 1/S in one fused custom-DVE op
            Q = work.tile([128, NL], F16, tag="Q", bufs=4, name=f"Q_{it}_{jt}")
            nc.vector._custom_dve(
                QRECIP, out=Q, in0=S16,
                in1=mask16[:, jt * NL:(jt + 1) * NL],
                s0=QRECIP_C0, s1=QRECIP_C1)
            # R8 = E8 * broadcast(Q) over the 8 channel blocks, one DVE op
            R8 = work.tile([128, E8W], F16, tag="R", bufs=PIPE_DEPTH + 1,
                           name=f"R8_{it}_{jt}")
            nc.vector.tensor_mul(
                out=R8.rearrange("p (c i) -> p c i", c=C),
                in0=E8.rearrange("p (c i) -> p c i", c=C),
                in1=Q.unsqueeze(1).broadcast_to([128, C, NL]))
            pending.append((jt, R8))
        for pjt, pR8 in pending:
            ph3_emit(pjt, pR8, 0, C)
        for t in range(NPAIR):
            # N=1 dummy stop: closes the sim accumulation group, no-op on HW
            nc.tensor.matmul(out=agg[t][:, 0:1], lhsT=ones16,
                             rhs=zeros16[:, 0:1], start=False, stop=True)
        # residual + renorm
        zpre = []
        for t in range(NPAIR):
            zq = work.tile([128, NL], F16, tag="zpre0", bufs=5,
                           name=f"zpre_{it}_{t}")
            nc.vector.tensor_add(out=zq, in0=zrows[t], in1=agg[t])
            zpre.append(zq)
        zrows, natrows = normalize_and_rows(zpre, it=it,
                                            want_nat=(it < ITERS - 1))
        if it < ITERS - 1:
            zfold8 = ship_all(zrows, natrows, it=it)

    # ================= output: h @ W_o + bias =================
    for ib in range(4):
        op = psum.tile([128, OUT], F32, tag="L", bufs=2, name=f"op_{ib}")
        for kt in range(4):
            nc.tensor.matmul(out=op,
                             lhsT=zrows[kt][:, ib * 128:(ib + 1) * 128],
                             rhs=wo16[:, kt * OUT:(kt + 1) * OUT],
                             start=(kt == 0), stop=False)
        nc.tensor.matmul(out=op, lhsT=ones16, rhs=bias16, start=False, stop=True)
        ot = work.tile([128, OUT], F32, tag="ot", bufs=2, name=f"ot_{ib}")
        nc.vector.tensor_copy(out=ot, in_=op)
        nc.sync.dma_start(out=outd[ib * 128:(ib + 1) * 128, :], in_=ot)

    ctx.close()


def _make_in_maps(features, adj, W, b, W_o, bias):
    import ml_dtypes
    features = np.asarray(features, dtype=np.float32)
    adj = np.asarray(adj, dtype=np.float32)
    W = np.asarray(W, dtype=np.float32)
    b = np.asarray(b, dtype=np.float32)
    W_o = np.asarray(W_o, dtype=np.float32)
    bias = np.asarray(bias, dtype=np.float32)

    f16 = np.float16
    f8 = ml_dtypes.float8_e4m3
    wall = np.ascontiguousarray(
        W.transpose(1, 0, 2).reshape(IN_DIM, CD)).astype(f16)
    bflat = np.ascontiguousarray(b.reshape(1, CD).reshape(NPAIR, 128).T).astype(np.float32)
    ident = np.eye(128, dtype=f16)
    blkd = np.zeros((128, NPAIR * 8), dtype=f16)
    seld = np.zeros((8, NPAIR * 128), dtype=f16)
    for t in range(NPAIR):
        for h in range(2):
            c = 2 * t + h
            blkd[h * 64:(h + 1) * 64, t * 8 + c] = 1.0
            seld[c, t * 128 + h * 64:t * 128 + (h + 1) * 64] = 1.0
    onesd = np.ones((1, 128), dtype=f16)
    wo16 = W_o.astype(f16)
    bias16 = bias.reshape(1, OUT).astype(f16)

    in_maps = []
    for r in range(NCORES):
        rows = slice(r * NL, (r + 1) * NL)
        in_maps.append({
            "featT": np.ascontiguousarray(features[rows].T).astype(f16),
            "wall": wall,
            "bflat": bflat,
            "maskT": np.ascontiguousarray(adj[rows].T).astype(f16),
            "wo": wo16,
            "biasd": bias16,
            "ident": ident,
            "blkd": blkd,
            "seld": seld,
            "onesd": onesd,
        })
    return in_maps


_NC_CACHE = []


def _get_nc():
    if not _NC_CACHE:
        _NC_CACHE.append(_build_nc())
    return _NC_CACHE[0]


def run(inputs, trace=False, **kwargs):
    nc = _get_nc()
    in_maps = _make_in_maps(**inputs)
    res = run_bass_kernel_spmd(nc, in_maps, core_ids=list(range(NCORES)),
                               trace=trace, **kwargs)
    out = np.concatenate([res.results[r]["outd"] for r in range(NCORES)],
                         axis=0).astype(np.float32)
    return out, res


def kernel(features, adj, W, b, W_o, bias):
    out, _ = run(dict(features=features, adj=adj, W=W, b=b, W_o=W_o, bias=bias))
    return out
